# revision 12
# baseline (speedup 1.0000x reference)
"""Trainium2 Bass kernel for nn_BoxLoss (YOLO-style box regression loss).

Contract: kernel(**inputs) takes FULL unsharded inputs (numpy), returns the
FULL scalar loss. Internally: pure data parallel over batch across 8
NeuronCores (4 images per core), each core computes its 12 (scale, image)
row losses entirely on-device, partial sums are all-reduced on-device and
the scalar is read back from core 0.

Only ~50 targets x 12 rows of work exist per core; the big [B,A,g,g,85]
activation tensors are touched ONLY via an indirect (gather) DMA of the
<=600 matched cells x 4 channels actually used by the loss - the kernel
never streams the full tensors.

Device layout: partition dim = target j (50), free dim = r = s*4 + b
(3 scales x 4 local images = 12 rows), innermost c (4 box channels)
where applicable.
"""

import numpy as np

import concourse.bass as bass
import concourse.bacc as bacc
import concourse.mybir as mybir
import concourse.tile as tile

NCORES = 8
GRIDS = (52, 26, 13)
A = 3           # anchors per scale
T = 50          # targets per image
PB = 4          # images per core
R = 3 * PB      # (scale, image) rows per core
BLOCK = 8192.0  # per-row key offset; cells < 3*52*52 = 8112 < 8192
SENT = 8112.0   # sentinel cell id for unmatched targets (>= any real cell)
B_TOTAL = 32

F32 = mybir.dt.float32
I32 = mybir.dt.int32

# element sizes of each scale's per-core slice of outcat
_SCALE_ELEMS = [PB * A * g * g * 85 for g in GRIDS]
_SCALE_BASE = [0, _SCALE_ELEMS[0], _SCALE_ELEMS[0] + _SCALE_ELEMS[1]]
OUTCAT_ELEMS = sum(_SCALE_ELEMS)


def _consts():
    """Inline constant tensors (shape/grid structure only)."""
    j = np.arange(T, dtype=np.float32)[:, None]          # [50,1]
    r = np.arange(R, dtype=np.float32)[None, :]          # [1,12]
    s = (r // PB).astype(np.int64)                       # scale of each row
    b = (r % PB).astype(np.int64)                        # local image of each row
    g = np.array(GRIDS, dtype=np.float32)[s]             # [1,12]
    ones50 = np.ones((T, 1), np.float32)

    jcr = np.broadcast_to(SENT + r * BLOCK, (T, R))
    g2_85 = np.broadcast_to(g * g * 85.0, (T, R))
    g85 = np.broadcast_to(g * 85.0, (T, R))
    base = np.array(_SCALE_BASE, dtype=np.float64)[s]
    bgoff = np.broadcast_to(base + b * (A * 85) * (g.astype(np.float64) ** 2),
                            (T, R)).astype(np.float32)
    hw4 = np.broadcast_to(g * g, (T, R))
    w4 = np.broadcast_to(g, (T, R))
    c12 = np.concatenate([jcr, g2_85, g85, bgoff, hw4, w4], axis=1).astype(np.float32)

    g4 = np.broadcast_to(g[:, :, None], (T, R, 4)).reshape(T, 48)
    mxy = np.broadcast_to(np.array([1, 1, 0, 0], np.float32), (T, R, 4)).reshape(T, 48)
    c48 = np.concatenate([g4, mxy], axis=1).astype(np.float32)

    eye = np.eye(T, dtype=np.float32)
    later = np.triu(np.ones((T, T), np.float32), 1)          # [j,k] = k>j
    later600 = np.broadcast_to(later[:, None, :], (T, R, T)).reshape(T, R * T)
    return c12, c48, eye, ones50, np.ascontiguousarray(later600)


def build_nc(use_collective: bool = True, debug_outputs: bool = False):
    nc = bacc.Bacc("TRN2", target_bir_lowering=False, debug=False,
                   num_devices=NCORES)

    tg16_d = nc.dram_tensor("tg16", [T, 16], F32, kind="ExternalInput")
    awh_d = nc.dram_tensor("awh", [1, 72], F32, kind="ExternalInput")
    outcat_d = nc.dram_tensor("outcat", [OUTCAT_ELEMS], F32, kind="ExternalInput")
    loss_d = nc.dram_tensor("loss", [1, 1], F32, kind="ExternalOutput")
    dbg = {}
    if debug_outputs:
        for nm, shape in [("d_key", [T, 12]), ("d_ov", [T, 12]),
                          ("d_m", [T, 12]), ("d_anc", [T, 12]),
                          ("d_overlap", [T, 12]), ("d_S", [T, 24]),
                          ("d_sums", [1, 24]), ("d_gath", [T, 48]),
                          ("d_idx", [T, 12]), ("d_t4", [T, 48]),
                          ("d_keyT", [R, T]), ("d_fxy", [T, 24])]:
            dbg[nm] = nc.dram_tensor(nm, shape, F32, kind="ExternalOutput")

    c12_np, c48_np, eye_np, ones_np, later_np = _consts()
    c12_d = nc.inline_tensor(c12_np, name="c12")
    c48_d = nc.inline_tensor(c48_np, name="c48")
    eye_d = nc.inline_tensor(eye_np, name="eye50")
    later_d = nc.inline_tensor(later_np, name="later")

    AL = mybir.AluOpType

    with tile.TileContext(nc) as tc:
        with (
            tc.tile_pool(name="sbuf", bufs=1) as sp,
            tc.tile_pool(name="psum", bufs=1, space="PSUM") as pp,
            tc.tile_pool(name="dram", bufs=1, space="DRAM") as dp,
        ):
            def tt(out, in0, in1, op):
                nc.vector.tensor_tensor(out=out, in0=in0, in1=in1, op=op)

            def ts(out, in0, s1, op, s2=None, op2=None):
                if op2 is None:
                    nc.vector.tensor_scalar(out=out, in0=in0, scalar1=s1,
                                            scalar2=None, op0=op)
                else:
                    nc.vector.tensor_scalar(out=out, in0=in0, scalar1=s1,
                                            scalar2=s2, op0=op, op1=op2)

            _tilen = [0]

            def new(shape, dt=F32, tag=None):
                _tilen[0] += 1
                return sp.tile(shape, dt, name=tag or f"t{_tilen[0]}")

            # ---------- loads ----------
            tgt = new([T, 16])
            nc.sync.dma_start(out=tgt[:], in_=tg16_d[:, :])
            awhT = new([T, 72])
            nc.sync.dma_start(out=awhT[:], in_=awh_d[:, :].to_broadcast([T, 72]))
            c12 = new([T, 72])
            nc.sync.dma_start(out=c12[:], in_=c12_d[:, :])
            c48 = new([T, 96])
            nc.sync.dma_start(out=c48[:], in_=c48_d[:, :])
            eye = new([T, T])
            nc.sync.dma_start(out=eye[:], in_=eye_d[:, :])

            JCR = c12[:, 0:12]
            G2_85 = c12[:, 12:24]
            G85 = c12[:, 24:36]
            BGOFF = c12[:, 36:48]
            HW4 = c12[:, 48:60]
            W4 = c12[:, 60:72]
            G4 = c48[:, 0:48]
            MXY = c48[:, 48:96]

            # ---------- t = raw * g ----------
            t4 = new([T, 48])
            tt(t4[:], tgt[:, None, :].to_broadcast([T, 3, 16]), G4, AL.mult)
            t4v = t4[:].rearrange("p (r c) -> p r c", c=4)
            txy = t4v[:, :, 0:2]            # [50,12,2] strided view
            twh = t4v[:, :, 2:4]

            # ---------- floor(xy) ----------
            r1 = new([T, 24])
            ts(r1[:], txy, float(2 ** 23), AL.add)
            r2 = new([T, 24])
            ts(r2[:], r1[:], -float(2 ** 23), AL.add)
            gtm = new([T, 24])
            tt(gtm[:], r2[:], txy, AL.is_gt)
            fxy = new([T, 24])
            tt(fxy[:], r2[:], gtm[:], AL.subtract)
            fv = fxy[:].rearrange("p (r c) -> p r c", c=2)
            cx4 = fv[:, :, 0:1]
            cy4 = fv[:, :, 1:2]

            # ---------- target rect (zt box) ----------
            ztxy = new([T, 24])
            tt(ztxy[:], txy, fxy[:], AL.subtract)
            zt05 = new([T, 24])
            ts(zt05[:], ztxy[:], -0.5, AL.add)
            twhh = new([T, 24])
            ts(twhh[:], twh, 0.5, AL.mult)
            zv = zt05[:].rearrange("p (r c) -> p r c", c=2)
            hv = twhh[:].rearrange("p (r c) -> p r c", c=2)
            ztx, zty = zv[:, :, 0:1], zv[:, :, 1:2]
            twx, twy = hv[:, :, 0:1], hv[:, :, 1:2]
            x0t = new([T, 12]); tt(x0t[:], ztx, twx, AL.subtract)
            x1t = new([T, 12]); tt(x1t[:], ztx, twx, AL.add)
            y0t = new([T, 12]); tt(y0t[:], zty, twy, AL.subtract)
            y1t = new([T, 12]); tt(y1t[:], zty, twy, AL.add)
            aa1 = new([T, 12]); tt(aa1[:], x1t[:], x0t[:], AL.subtract)
            aa2 = new([T, 12]); tt(aa2[:], y1t[:], y0t[:], AL.subtract)
            areat = new([T, 12]); tt(areat[:], aa1[:], aa2[:], AL.mult)

            # ---------- anchor rects ----------
            awhh = new([T, 72])
            ts(awhh[:], awhT[:], 0.5, AL.mult)
            awh_w = awhh[:, 0:36]
            awh_h = awhh[:, 36:72]
            areaa = new([T, 36])
            tt(areaa[:], awhT[:, 0:36], awhT[:, 36:72], AL.mult)
            x0a = new([T, 36]); ts(x0a[:], awh_w, -1.0, AL.mult)
            y0a = new([T, 36]); ts(y0a[:], awh_h, -1.0, AL.mult)

            def bc3(ap12):
                return ap12[:, :, 0:1].to_broadcast([T, 12, 3])

            # ---------- IoU [50, 36] ----------
            x0 = new([T, 36]); tt(x0[:], bc3(x0t[:].rearrange("p (r o) -> p r o", o=1)), x0a[:], AL.max)
            x1 = new([T, 36]); tt(x1[:], bc3(x1t[:].rearrange("p (r o) -> p r o", o=1)), awh_w, AL.min)
            y0 = new([T, 36]); tt(y0[:], bc3(y0t[:].rearrange("p (r o) -> p r o", o=1)), y0a[:], AL.max)
            y1 = new([T, 36]); tt(y1[:], bc3(y1t[:].rearrange("p (r o) -> p r o", o=1)), awh_h, AL.min)
            dx = new([T, 36]); tt(dx[:], x1[:], x0[:], AL.subtract)
            dy = new([T, 36]); tt(dy[:], y1[:], y0[:], AL.subtract)
            fx = new([T, 36]); ts(fx[:], dx[:], 0.0, AL.is_gt)
            fy = new([T, 36]); ts(fy[:], dy[:], 0.0, AL.is_gt)
            flg = new([T, 36]); tt(flg[:], fx[:], fy[:], AL.mult)
            it1 = new([T, 36]); tt(it1[:], dx[:], dy[:], AL.mult)
            inter = new([T, 36]); tt(inter[:], it1[:], flg[:], AL.mult)
            un1 = new([T, 36])
            tt(un1[:], bc3(areat[:].rearrange("p (r o) -> p r o", o=1)), areaa[:], AL.add)
            union = new([T, 36]); tt(union[:], un1[:], inter[:], AL.subtract)
            runi = new([T, 36])
            nc.vector.reciprocal(out=runi[:], in_=union[:])
            iou = new([T, 36]); tt(iou[:], inter[:], runi[:], AL.mult)

            # ---------- overlap / argmax / match ----------
            overlap = new([T, 12])
            nc.vector.reduce_max(out=overlap[:],
                                 in_=iou[:].rearrange("p (r a) -> p r a", a=3),
                                 axis=mybir.AxisListType.X)
            iv = iou[:].rearrange("p (r a) -> p r a", a=3)
            eq0 = new([T, 12]); tt(eq0[:], iv[:, :, 0:1], overlap[:], AL.is_equal)
            eq1 = new([T, 12]); tt(eq1[:], iv[:, :, 1:2], overlap[:], AL.is_equal)
            t2 = new([T, 12]); ts(t2[:], eq1[:], 0.0, AL.is_equal, 1.0, AL.add)
            neq0 = new([T, 12]); ts(neq0[:], eq0[:], 0.0, AL.is_equal)
            anc = new([T, 12]); tt(anc[:], neq0[:], t2[:], AL.mult)

            sv = new([T, 4])
            nc.vector.reduce_sum(out=sv[:],
                                 in_=tgt[:].rearrange("p (b c) -> p b c", c=4),
                                 axis=mybir.AxisListType.X)
            v4 = new([T, 4]); ts(v4[:], sv[:], 0.0, AL.is_gt)
            om = new([T, 12]); ts(om[:], overlap[:], 0.5, AL.is_gt)
            m = new([T, 12])
            tt(m[:], om[:].rearrange("p (s b) -> p s b", b=4),
               v4[:, None, :].to_broadcast([T, 3, 4]), AL.mult)

            # ---------- cell id + dedup key ----------
            ca = new([T, 12]); tt(ca[:], anc[:], HW4, AL.mult)
            cb = new([T, 12]); tt(cb[:], cy4, W4, AL.mult)
            cc = new([T, 12]); tt(cc[:], ca[:], cb[:], AL.add)
            cell = new([T, 12]); tt(cell[:], cc[:], cx4, AL.add)
            k1 = new([T, 12]); ts(k1[:], cell[:], -SENT, AL.add)
            k2 = new([T, 12]); tt(k2[:], k1[:], m[:], AL.mult)
            key = new([T, 12]); tt(key[:], k2[:], JCR, AL.add)

            # transpose key via PE identity matmul -> [12, 50]
            keyT_p = pp.tile([R, T], F32)
            nc.tensor.matmul(out=keyT_p[:], lhsT=key[:], rhs=eye[:],
                             start=True, stop=True)
            keyT = new([R, T])
            nc.vector.tensor_copy(out=keyT[:], in_=keyT_p[:])
            kd2 = nc.dram_tensor("kd2", [R * T], F32)
            nc.sync.dma_start(out=kd2[:].rearrange("(r k) -> r k", k=T), in_=keyT[:])
            keyB = new([T, R * T])
            nc.sync.dma_start(out=keyB[:], in_=kd2[:].unsqueeze(0).to_broadcast([T, R * T]))

            lat = new([T, R * T])
            nc.sync.dma_start(out=lat[:], in_=later_d[:, :])
            E = new([T, R * T])
            tt(E[:], key[:, :, None].to_broadcast([T, 12, T]), keyB[:], AL.is_equal)
            # keep only k > j comparisons
            EL = new([T, R * T])
            tt(EL[:], E[:], lat[:], AL.mult)
            ov = new([T, 12])
            nc.vector.reduce_max(out=ov[:],
                                 in_=EL[:].rearrange("p (r k) -> p r k", k=T),
                                 axis=mybir.AxisListType.X)
            S = new([T, 24])
            nov = new([T, 12]); ts(nov[:], ov[:], 0.0, AL.is_equal)
            tt(S[:, 0:12], m[:], nov[:], AL.mult)          # winner

            # ---------- gather offsets ----------
            i1 = new([T, 12]); tt(i1[:], anc[:], G2_85, AL.mult)
            i2 = new([T, 12]); tt(i2[:], cy4, G85, AL.mult)
            i3 = new([T, 12]); tt(i3[:], i1[:], i2[:], AL.add)
            i4 = new([T, 12]); ts(i4[:], cx4, 85.0, AL.mult)
            i5 = new([T, 12]); tt(i5[:], i3[:], i4[:], AL.add)
            idxf = new([T, 12]); tt(idxf[:], i5[:], BGOFF, AL.add)
            idxi = new([T, 12], I32)
            nc.vector.tensor_copy(out=idxi[:], in_=idxf[:])
            # HW indirect gather consumes ONE index per dst partition and
            # reads the dst row's free size consecutively -> one gather per r.
            gath = new([T, 48])
            for r in range(R):
                nc.gpsimd.indirect_dma_start(
                    out=gath[:, r * 4:(r + 1) * 4], out_offset=None,
                    in_=outcat_d[:].unsqueeze(1),
                    in_offset=bass.IndirectOffsetOnAxis(ap=idxi[:, r:r + 1], axis=0),
                )

            # ---------- loss terms ----------
            rct = new([T, 48]); nc.vector.reciprocal(out=rct[:], in_=t4[:])
            rst = new([T, 48]); nc.scalar.sqrt(out=rst[:], in_=rct[:])
            rcp = new([T, 48]); nc.vector.reciprocal(out=rcp[:], in_=gath[:])
            rsp = new([T, 48]); nc.scalar.sqrt(out=rsp[:], in_=rcp[:])
            dP = new([T, 48]); tt(dP[:], gath[:], t4[:], AL.subtract)
            dR = new([T, 48]); tt(dR[:], rsp[:], rst[:], AL.subtract)
            dd = new([T, 48]); tt(dd[:], dP[:], dR[:], AL.subtract)
            md = new([T, 48]); tt(md[:], MXY, dd[:], AL.mult)
            sel = new([T, 48]); tt(sel[:], dR[:], md[:], AL.add)
            sq = new([T, 48]); tt(sq[:], sel[:], sel[:], AL.mult)
            termsum = new([T, 12])
            nc.vector.reduce_sum(out=termsum[:],
                                 in_=sq[:].rearrange("p (r c) -> p r c", c=4),
                                 axis=mybir.AxisListType.X)
            tt(S[:, 12:24], S[:, 0:12], termsum[:], AL.mult)

            # ---------- per-row normalize + total ----------
            ones = new([T, 1])
            nc.vector.memset(ones[:], 1.0)
            sums_p = pp.tile([1, 24], F32)
            nc.tensor.matmul(out=sums_p[:], lhsT=ones[:], rhs=S[:],
                             start=True, stop=True)
            sums = new([1, 24])
            nc.vector.tensor_copy(out=sums[:], in_=sums_p[:])
            mx2 = new([1, 12])
            ts(mx2[:], sums[:, 0:12], 1.0, AL.max, 2.0, AL.mult)
            rden = new([1, 12]); nc.vector.reciprocal(out=rden[:], in_=mx2[:])
            rl = new([1, 12]); tt(rl[:], sums[:, 12:24], rden[:], AL.mult)
            part = new([1, 1])
            nc.vector.reduce_sum(out=part[:], in_=rl[:], axis=mybir.AxisListType.X)
            p32 = new([1, 1])
            ts(p32[:], part[:], 1.0 / B_TOTAL, AL.mult)

            if debug_outputs:
                for nm, src in [("d_key", key), ("d_ov", ov), ("d_m", m),
                                ("d_anc", anc), ("d_overlap", overlap),
                                ("d_S", S), ("d_sums", sums),
                                ("d_gath", gath), ("d_idx", idxf),
                                ("d_t4", t4), ("d_keyT", keyT),
                                ("d_fxy", fxy)]:
                    nc.sync.dma_start(out=dbg[nm][:, :], in_=src[:])

            if use_collective:
                ccin = dp.tile([1, 1], F32)
                ccout = dp.tile([1, 1], F32)
                nc.sync.dma_start(out=ccin[:], in_=p32[:])
                nc.gpsimd.collective_compute(
                    "AllReduce", AL.add,
                    replica_groups=[list(range(NCORES))],
                    ins=[ccin[:].opt()], outs=[ccout[:].opt()],
                )
                nc.sync.dma_start(out=loss_d[:, :], in_=ccout[:])
            else:
                nc.sync.dma_start(out=loss_d[:, :], in_=p32[:])

    nc.compile()
    return nc


def make_in_maps(output0, anchors0, output1, anchors1, output2, anchors2,
                 targets):
    outs = [np.asarray(output0), np.asarray(output1), np.asarray(output2)]
    ancs = [np.asarray(anchors0), np.asarray(anchors1), np.asarray(anchors2)]
    tg = np.asarray(targets)

    aw = np.concatenate([np.tile(a[:, 0], PB) for a in ancs])  # [36] (r, a)
    ah = np.concatenate([np.tile(a[:, 1], PB) for a in ancs])
    awh = np.concatenate([aw, ah]).astype(np.float32)[None, :]  # [1,72]

    in_maps = []
    for c in range(NCORES):
        sl = slice(c * PB, (c + 1) * PB)
        tg16 = np.ascontiguousarray(
            tg[sl, :, 1:5].transpose(1, 0, 2).reshape(T, 16).astype(np.float32))
        outcat = np.concatenate([o[sl].ravel() for o in outs]).astype(np.float32)
        in_maps.append({"tg16": tg16, "awh": awh, "outcat": outcat})
    return in_maps


_NC_CACHE = {}


def kernel(output0, anchors0, output1, anchors1, output2, anchors2, targets):
    from concourse.bass_utils import run_bass_kernel_spmd

    if "nc" not in _NC_CACHE:
        _NC_CACHE["nc"] = build_nc(use_collective=True)
    nc = _NC_CACHE["nc"]
    in_maps = make_in_maps(output0, anchors0, output1, anchors1, output2,
                           anchors2, targets)
    res = run_bass_kernel_spmd(nc, in_maps, list(range(NCORES)))
    out = res.results[0]["loss"]
    return np.float32(out.reshape(())).astype(np.float32)


# revision 13
# speedup vs baseline: 1.9502x; 1.9502x over previous
"""Trainium2 Bass kernel for nn_BoxLoss (YOLO-style box regression loss).

Contract: kernel(**inputs) takes FULL unsharded inputs (numpy), returns the
FULL scalar loss. Internally: pure data parallel over batch across 8
NeuronCores (4 images per core); each core computes its 12 (scale, image)
row losses entirely on-device and writes its partial sum; the host adds
the 8 partials while unsharding.

Only ~50 targets x 12 rows of real work exist per core; the big
[B,A,g,g,85] activation tensors are touched ONLY via indirect (gather)
DMAs of the <=600 matched cells x 4 channels the loss actually reads -
the kernel never streams the full tensors.

Device layout: partition dim = target j (50), free dim r = s*4 + b
(3 scales x 4 local images = 12 rows), innermost c (4 box channels).
The gather runs in a [100, *] layout (partition = (b-half, j)) so each
indirect DMA consumes one index per partition (the HW semantics).
"""

import numpy as np

import concourse.bass as bass
import concourse.bacc as bacc
import concourse.mybir as mybir
import concourse.tile as tile

NCORES = 8
GRIDS = (52, 26, 13)
A = 3           # anchors per scale
T = 50          # targets per image
PB = 4          # images per core
R = 3 * PB      # (scale, image) rows per core
BLOCK = 8192.0  # per-row key offset; cells < 3*52*52 = 8112 < 8192
SENT = 8112.0   # sentinel cell id for unmatched targets (>= any real cell)
B_TOTAL = 32

F32 = mybir.dt.float32
I32 = mybir.dt.int32

_SCALE_ELEMS = [PB * A * g * g * 85 for g in GRIDS]
_SCALE_BASE = [0, _SCALE_ELEMS[0], _SCALE_ELEMS[0] + _SCALE_ELEMS[1]]
OUTCAT_ELEMS = sum(_SCALE_ELEMS)

# merged-constant column layout
_C_JCR = 0        # [0,12)   8112 + r*8192
_C_G285 = 12      # [12,24)  g^2*85
_C_G85 = 24       # [24,36)  g*85
_C_BGOFF = 36     # [36,48)  scale base + b*3*g^2*85
_C_HW = 48        # [48,60)  g^2
_C_W = 60         # [60,72)  g
_C_G4 = 72        # [72,120) g per (s,b,c)
_C_MXY = 120      # [120,168) 1 for xy channels
_C_EYE = 168      # [168,218) identity 50x50
_C_LATER = 218    # [218,818) dedup mask (j,k>j) per r
_C_TOT = 818


def _consts():
    j = np.arange(T, dtype=np.float32)[:, None]
    r = np.arange(R, dtype=np.float32)[None, :]
    s = (r // PB).astype(np.int64)
    b = (r % PB).astype(np.int64)
    g = np.array(GRIDS, dtype=np.float32)[s]

    jcr = np.broadcast_to(SENT + r * BLOCK, (T, R))
    g2_85 = np.broadcast_to(g * g * 85.0, (T, R))
    g85 = np.broadcast_to(g * 85.0, (T, R))
    base = np.array(_SCALE_BASE, dtype=np.float64)[s]
    bgoff = np.broadcast_to(base + b * (A * 85) * (g.astype(np.float64) ** 2),
                            (T, R)).astype(np.float32)
    hw4 = np.broadcast_to(g * g, (T, R))
    w4 = np.broadcast_to(g, (T, R))
    g4 = np.broadcast_to(g[:, :, None], (T, R, 4)).reshape(T, 48)
    mxy = np.broadcast_to(np.array([1, 1, 0, 0], np.float32), (T, R, 4)).reshape(T, 48)
    eye = np.eye(T, dtype=np.float32)
    later = np.triu(np.ones((T, T), np.float32), 1)
    later600 = np.broadcast_to(later[:, None, :], (T, R, T)).reshape(T, R * T)
    cat = np.concatenate([jcr, g2_85, g85, bgoff, hw4, w4, g4, mxy, eye,
                          later600], axis=1).astype(np.float32)
    assert cat.shape == (T, _C_TOT)
    return np.ascontiguousarray(cat)


def build_nc(use_collective: bool = False, debug_outputs: bool = False):
    nc = bacc.Bacc("TRN2", target_bir_lowering=False, debug=False,
                   num_devices=NCORES)

    tg16_d = nc.dram_tensor("tg16", [T, 16], F32, kind="ExternalInput")
    awh_d = nc.dram_tensor("awh", [1, 72], F32, kind="ExternalInput")
    outcat_d = nc.dram_tensor("outcat", [OUTCAT_ELEMS], F32, kind="ExternalInput")
    loss_d = nc.dram_tensor("loss", [1, 1], F32, kind="ExternalOutput")
    cst_d = nc.inline_tensor(_consts(), name="cst")

    dbg = {}
    if debug_outputs:
        for nm, shape in [("d_key", [T, 12]), ("d_ov", [T, 12]),
                          ("d_m", [T, 12]), ("d_anc", [T, 12]),
                          ("d_overlap", [T, 12]), ("d_S", [T, 24]),
                          ("d_sums", [1, 24]), ("d_gath", [T, 48]),
                          ("d_idx", [T, 12]), ("d_t4", [T, 48]),
                          ("d_fxy", [T, 24])]:
            dbg[nm] = nc.dram_tensor(nm, shape, F32, kind="ExternalOutput")

    AL = mybir.AluOpType
    AX = mybir.AxisListType.X

    with tile.TileContext(nc) as tc:
        with (
            tc.tile_pool(name="sbuf", bufs=1) as sp,
            tc.tile_pool(name="psum", bufs=1, space="PSUM") as pp,
            tc.tile_pool(name="dram", bufs=1, space="DRAM") as dp,
        ):
            def tt(out, in0, in1, op):
                nc.vector.tensor_tensor(out=out, in0=in0, in1=in1, op=op)

            def ts(out, in0, s1, op, s2=None, op2=None):
                if op2 is None:
                    nc.vector.tensor_scalar(out=out, in0=in0, scalar1=s1,
                                            scalar2=None, op0=op)
                else:
                    nc.vector.tensor_scalar(out=out, in0=in0, scalar1=s1,
                                            scalar2=s2, op0=op, op1=op2)

            def stt(out, in0, scalar, in1, op0, op1):
                nc.vector.scalar_tensor_tensor(out=out, in0=in0, scalar=scalar,
                                               in1=in1, op0=op0, op1=op1)

            _tn = [0]

            def new(shape, dt=F32):
                _tn[0] += 1
                return sp.tile(shape, dt, name=f"t{_tn[0]}")

            # ---------- loads ----------
            tgt = new([T, 16])
            nc.sync.dma_start(out=tgt[:], in_=tg16_d[:, :])
            awhT = new([T, 72])
            nc.sync.dma_start(out=awhT[:], in_=awh_d[:, :].to_broadcast([T, 72]))
            cst = new([T, _C_TOT])
            nc.sync.dma_start(out=cst[:], in_=cst_d[:, :])

            def C(c0, w):
                return cst[:, c0:c0 + w]

            # ---------- t = raw * g ----------
            t4 = new([T, 48])
            tt(t4[:], tgt[:, None, :].to_broadcast([T, 3, 16]), C(_C_G4, 48),
               AL.mult)
            t4v = t4[:].rearrange("p (r c) -> p r c", c=4)
            txy = t4v[:, :, 0:2]            # [50,12,2] strided
            twh = t4v[:, :, 2:4]

            # ---------- floor(xy): round-magic + fixup ----------
            r1 = new([T, 24])
            ts(r1[:], txy, float(2 ** 23), AL.add)
            r2 = new([T, 24])
            ts(r2[:], r1[:], -float(2 ** 23), AL.add)
            gtm = new([T, 24])
            tt(gtm[:], r2[:], txy, AL.is_gt)
            fxy = new([T, 24])
            tt(fxy[:], r2[:], gtm[:], AL.subtract)
            fv = fxy[:].rearrange("p (r q) -> p r q", q=2)
            cx4 = fv[:, :, 0:1]
            cy4 = fv[:, :, 1:2]

            # ---------- target rect ----------
            zt05 = new([T, 24])     # (txy - 0.5) - fxy
            stt(zt05[:], txy, -0.5, fxy[:], AL.add, AL.subtract)
            twhh = new([T, 24])
            ts(twhh[:], twh, 0.5, AL.mult)
            lo = new([T, 24]); tt(lo[:], zt05[:], twhh[:], AL.subtract)
            hi = new([T, 24]); tt(hi[:], zt05[:], twhh[:], AL.add)
            dT = new([T, 24]); tt(dT[:], hi[:], lo[:], AL.subtract)
            dv = dT[:].rearrange("p (r q) -> p r q", q=2)
            areat = new([T, 12]); tt(areat[:], dv[:, :, 0:1], dv[:, :, 1:2], AL.mult)

            # ---------- anchor rects + IoU in (q, r, a) layout ----------
            awhh = new([T, 72]); ts(awhh[:], awhT[:], 0.5, AL.mult)
            nawhh = new([T, 72]); ts(nawhh[:], awhh[:], -1.0, AL.mult)
            areaa = new([T, 36])
            tt(areaa[:], awhT[:, 0:36], awhT[:, 36:72], AL.mult)

            def bc72(t24):
                # [50,24] (r,q) -> [50, (q,r,a)] = [50,72]
                return (t24[:].rearrange("p (r q) -> p q r", q=2)[:, :, :, None]
                        .to_broadcast([T, 2, 12, 3]))

            P0 = new([T, 72]); tt(P0[:], bc72(lo), nawhh[:], AL.max)
            P1 = new([T, 72]); tt(P1[:], bc72(hi), awhh[:], AL.min)
            D = new([T, 72]); tt(D[:], P1[:], P0[:], AL.subtract)
            FG = new([T, 72]); ts(FG[:], D[:], 0.0, AL.is_gt)
            flag = new([T, 36]); tt(flag[:], FG[:, 0:36], FG[:, 36:72], AL.mult)
            pre = new([T, 36]); tt(pre[:], D[:, 0:36], D[:, 36:72], AL.mult)
            inter = new([T, 36]); tt(inter[:], pre[:], flag[:], AL.mult)
            un1 = new([T, 36])
            tt(un1[:], areat[:, :, None].to_broadcast([T, 12, 3]), areaa[:], AL.add)
            union = new([T, 36]); tt(union[:], un1[:], inter[:], AL.subtract)
            runi = new([T, 36]); nc.vector.reciprocal(out=runi[:], in_=union[:])
            iou = new([T, 36]); tt(iou[:], inter[:], runi[:], AL.mult)

            # ---------- overlap / argmax / match ----------
            overlap = new([T, 12])
            nc.vector.reduce_max(out=overlap[:],
                                 in_=iou[:].rearrange("p (r a) -> p r a", a=3),
                                 axis=AX)
            iv = iou[:].rearrange("p (r a) -> p r a", a=3)
            eq0 = new([T, 12]); tt(eq0[:], iv[:, :, 0:1], overlap[:], AL.is_equal)
            eq1 = new([T, 12]); tt(eq1[:], iv[:, :, 1:2], overlap[:], AL.is_equal)
            t2 = new([T, 12]); ts(t2[:], eq1[:], 0.0, AL.is_equal, 1.0, AL.add)
            neq0 = new([T, 12]); ts(neq0[:], eq0[:], 0.0, AL.is_equal)
            anc = new([T, 12]); tt(anc[:], neq0[:], t2[:], AL.mult)

            sv = new([T, 4])
            nc.vector.reduce_sum(out=sv[:],
                                 in_=tgt[:].rearrange("p (b c) -> p b c", c=4),
                                 axis=AX)
            v4 = new([T, 4]); ts(v4[:], sv[:], 0.0, AL.is_gt)
            om = new([T, 12]); ts(om[:], overlap[:], 0.5, AL.is_gt)
            m = new([T, 12])
            tt(m[:], om[:].rearrange("p (s b) -> p s b", b=4),
               v4[:, None, :].to_broadcast([T, 3, 4]), AL.mult)

            # ---------- cell id + gather offsets (emitted first: unblocks
            # the gathers as early as possible) ----------
            ca = new([T, 12]); tt(ca[:], anc[:], C(_C_HW, 12), AL.mult)
            cb = new([T, 12]); tt(cb[:], cy4, C(_C_W, 12), AL.mult)
            cc = new([T, 12]); tt(cc[:], ca[:], cb[:], AL.add)
            cell = new([T, 12]); tt(cell[:], cc[:], cx4, AL.add)
            idf = new([T, 12])   # (cell*85) + BGOFF
            stt(idf[:], cell[:], 85.0, C(_C_BGOFF, 12), AL.mult, AL.add)
            idxi = new([T, 12], I32)
            nc.vector.tensor_copy(out=idxi[:], in_=idf[:])

            # rearrange indices to [100,6]: partition p = bh*50+j,
            # col q = s*2+bl  (r = s*4 + bh*2 + bl)
            idx2 = new([100, 6], I32)
            iview = idxi[:].rearrange("p (s bh bl) -> p s bh bl", bh=2, bl=2)
            for bh in range(2):
                nc.sync.dma_start(out=idx2[bh * 50:(bh + 1) * 50, :],
                                  in_=iview[:, :, bh, :])

            # HW indirect gather: ONE index per dst partition, dst row read
            # consecutively -> 6 gathers of [100, 4].
            gath2 = new([100, 24])
            for q in range(6):
                nc.gpsimd.indirect_dma_start(
                    out=gath2[:, q * 4:(q + 1) * 4], out_offset=None,
                    in_=outcat_d[:].unsqueeze(1),
                    in_offset=bass.IndirectOffsetOnAxis(ap=idx2[:, q:q + 1],
                                                        axis=0),
                )
            gath = new([T, 48])
            gview = gath[:].rearrange("p (s u) -> p s u", u=16)
            for bh in range(2):
                nc.sync.dma_start(out=gview[:, :, bh * 8:(bh + 1) * 8],
                                  in_=gath2[bh * 50:(bh + 1) * 50, :])

            # ---------- dedup key + last-wins winner mask ----------
            kk = new([T, 12])    # (cell - SENT) * m
            stt(kk[:], cell[:], -SENT, m[:], AL.add, AL.mult)
            key = new([T, 12]); tt(key[:], kk[:], C(_C_JCR, 12), AL.add)

            keyT_p = pp.tile([R, T], F32, name="keyT_p")
            nc.tensor.matmul(out=keyT_p[:], lhsT=key[:], rhs=C(_C_EYE, T),
                             start=True, stop=True)
            keyT = new([R, T])
            nc.vector.tensor_copy(out=keyT[:], in_=keyT_p[:])
            kd2 = nc.dram_tensor("kd2", [R * T], F32)
            nc.sync.dma_start(out=kd2[:].rearrange("(r k) -> r k", k=T),
                              in_=keyT[:])
            keyB = new([T, R * T])
            nc.sync.dma_start(out=keyB[:],
                              in_=kd2[:].unsqueeze(0).to_broadcast([T, R * T]))

            E = new([T, R * T])
            tt(E[:], key[:, :, None].to_broadcast([T, 12, T]), keyB[:],
               AL.is_equal)
            EL = new([T, R * T])
            tt(EL[:], E[:], C(_C_LATER, R * T), AL.mult)
            ov = new([T, 12])
            nc.vector.reduce_max(out=ov[:],
                                 in_=EL[:].rearrange("p (r k) -> p r k", k=T),
                                 axis=AX)
            S = new([T, 24])
            nov = new([T, 12]); ts(nov[:], ov[:], 0.0, AL.is_equal)
            tt(S[:, 0:12], m[:], nov[:], AL.mult)          # winner

            # ---------- loss terms ----------
            rct = new([T, 48]); nc.vector.reciprocal(out=rct[:], in_=t4[:])
            rst = new([T, 48]); nc.scalar.sqrt(out=rst[:], in_=rct[:])
            rcp = new([T, 48]); nc.vector.reciprocal(out=rcp[:], in_=gath[:])
            rsp = new([T, 48]); nc.scalar.sqrt(out=rsp[:], in_=rcp[:])
            dP = new([T, 48]); tt(dP[:], gath[:], t4[:], AL.subtract)
            dR = new([T, 48]); tt(dR[:], rsp[:], rst[:], AL.subtract)
            dd = new([T, 48]); tt(dd[:], dP[:], dR[:], AL.subtract)
            md = new([T, 48]); tt(md[:], C(_C_MXY, 48), dd[:], AL.mult)
            sel = new([T, 48]); tt(sel[:], dR[:], md[:], AL.add)
            sq = new([T, 48]); tt(sq[:], sel[:], sel[:], AL.mult)
            termsum = new([T, 12])
            nc.vector.reduce_sum(out=termsum[:],
                                 in_=sq[:].rearrange("p (r c) -> p r c", c=4),
                                 axis=AX)
            tt(S[:, 12:24], S[:, 0:12], termsum[:], AL.mult)

            # ---------- per-row normalize + total ----------
            ones = new([T, 1])
            nc.vector.memset(ones[:], 1.0)
            sums_p = pp.tile([1, 24], F32, name="sums_p")
            nc.tensor.matmul(out=sums_p[:], lhsT=ones[:], rhs=S[:],
                             start=True, stop=True)
            sums = new([1, 24])
            nc.vector.tensor_copy(out=sums[:], in_=sums_p[:])
            mx2 = new([1, 12])
            ts(mx2[:], sums[:, 0:12], 1.0, AL.max, 2.0, AL.mult)
            rden = new([1, 12]); nc.vector.reciprocal(out=rden[:], in_=mx2[:])
            rl = new([1, 12]); tt(rl[:], sums[:, 12:24], rden[:], AL.mult)
            part = new([1, 1])
            nc.vector.reduce_sum(out=part[:], in_=rl[:], axis=AX)
            p32 = new([1, 1])
            ts(p32[:], part[:], 1.0 / B_TOTAL, AL.mult)

            if debug_outputs:
                for nm, src in [("d_key", key), ("d_ov", ov), ("d_m", m),
                                ("d_anc", anc), ("d_overlap", overlap),
                                ("d_S", S), ("d_sums", sums),
                                ("d_gath", gath), ("d_idx", idf),
                                ("d_t4", t4), ("d_fxy", fxy)]:
                    nc.sync.dma_start(out=dbg[nm][:, :], in_=src[:])

            if use_collective:
                ccin = dp.tile([1, 1], F32, name="ccin")
                ccout = dp.tile([1, 1], F32, name="ccout")
                nc.sync.dma_start(out=ccin[:], in_=p32[:])
                nc.gpsimd.collective_compute(
                    "AllReduce", AL.add,
                    replica_groups=[list(range(NCORES))],
                    ins=[ccin[:].opt()], outs=[ccout[:].opt()],
                )
                nc.sync.dma_start(out=loss_d[:, :], in_=ccout[:])
            else:
                nc.sync.dma_start(out=loss_d[:, :], in_=p32[:])

    nc.compile()
    return nc


def make_in_maps(output0, anchors0, output1, anchors1, output2, anchors2,
                 targets):
    outs = [np.asarray(output0), np.asarray(output1), np.asarray(output2)]
    ancs = [np.asarray(anchors0), np.asarray(anchors1), np.asarray(anchors2)]
    tg = np.asarray(targets)

    aw = np.concatenate([np.tile(a[:, 0], PB) for a in ancs])  # [36] (r, a)
    ah = np.concatenate([np.tile(a[:, 1], PB) for a in ancs])
    awh = np.concatenate([aw, ah]).astype(np.float32)[None, :]  # [1,72]

    in_maps = []
    for c in range(NCORES):
        sl = slice(c * PB, (c + 1) * PB)
        tg16 = np.ascontiguousarray(
            tg[sl, :, 1:5].transpose(1, 0, 2).reshape(T, 16).astype(np.float32))
        outcat = np.concatenate([o[sl].ravel() for o in outs]).astype(np.float32)
        in_maps.append({"tg16": tg16, "awh": awh, "outcat": outcat})
    return in_maps


_NC_CACHE = {}


def kernel(output0, anchors0, output1, anchors1, output2, anchors2, targets):
    from concourse.bass_utils import run_bass_kernel_spmd

    if "nc" not in _NC_CACHE:
        _NC_CACHE["nc"] = build_nc(use_collective=False)
    nc = _NC_CACHE["nc"]
    in_maps = make_in_maps(output0, anchors0, output1, anchors1, output2,
                           anchors2, targets)
    res = run_bass_kernel_spmd(nc, in_maps, list(range(NCORES)))
    total = np.float32(0.0)
    for c in range(NCORES):
        total += np.float32(res.results[c]["loss"].reshape(()))
    return np.float32(total)


# revision 14
# speedup vs baseline: 1.9764x; 1.0134x over previous
"""Trainium2 Bass kernel for nn_BoxLoss (YOLO-style box regression loss).

Contract: kernel(**inputs) takes FULL unsharded inputs (numpy), returns the
FULL scalar loss. Internally: pure data parallel over batch across 8
NeuronCores (4 images per core); each core computes its 12 (scale, image)
row losses entirely on-device and writes its partial sum; the host adds
the 8 partials while unsharding.

Only ~50 targets x 12 rows of real work exist per core; the big
[B,A,g,g,85] activation tensors are touched ONLY via indirect (gather)
DMAs of the <=600 matched cells x 4 channels the loss actually reads -
the kernel never streams the full tensors.

Device layout: partition dim = target j (50), free dim r = s*4 + b
(3 scales x 4 local images = 12 rows), innermost c (4 box channels).
The gather runs in a [100, *] layout (partition = (b-half, j)) so each
indirect DMA consumes one index per partition (the HW semantics).
"""

import numpy as np

import concourse.bass as bass
import concourse.bacc as bacc
import concourse.mybir as mybir
import concourse.tile as tile

NCORES = 8
GRIDS = (52, 26, 13)
A = 3           # anchors per scale
T = 50          # targets per image
PB = 4          # images per core
R = 3 * PB      # (scale, image) rows per core
BLOCK = 8192.0  # per-row key offset; cells < 3*52*52 = 8112 < 8192
SENT = 8112.0   # sentinel cell id for unmatched targets (>= any real cell)
B_TOTAL = 32

F32 = mybir.dt.float32
I32 = mybir.dt.int32

_SCALE_ELEMS = [PB * A * g * g * 85 for g in GRIDS]
_SCALE_BASE = [0, _SCALE_ELEMS[0], _SCALE_ELEMS[0] + _SCALE_ELEMS[1]]
OUTCAT_ELEMS = sum(_SCALE_ELEMS)

# cstA column layout
_C_JCR = 0        # [0,12)    8112 + r*8192
_C_BGOFF = 12     # [12,24)   scale base + b*3*g^2*85
_C_HW = 24        # [24,36)   g^2
_C_W = 36         # [36,48)   g
_C_G4 = 48        # [48,96)   g per (s,b,c)
_C_MXY = 96       # [96,144)  1 for xy channels
_C_EYE = 144      # [144,194) identity 50x50
_CA_TOT = 194


def _consts():
    r = np.arange(R, dtype=np.float32)[None, :]
    s = (r // PB).astype(np.int64)
    b = (r % PB).astype(np.int64)
    g = np.array(GRIDS, dtype=np.float32)[s]

    jcr = np.broadcast_to(SENT + r * BLOCK, (T, R))
    base = np.array(_SCALE_BASE, dtype=np.float64)[s]
    bgoff = np.broadcast_to(base + b * (A * 85) * (g.astype(np.float64) ** 2),
                            (T, R)).astype(np.float32)
    hw4 = np.broadcast_to(g * g, (T, R))
    w4 = np.broadcast_to(g, (T, R))
    g4 = np.broadcast_to(g[:, :, None], (T, R, 4)).reshape(T, 48)
    mxy = np.broadcast_to(np.array([1, 1, 0, 0], np.float32), (T, R, 4)).reshape(T, 48)
    eye = np.eye(T, dtype=np.float32)
    cstA = np.concatenate([jcr, bgoff, hw4, w4, g4, mxy, eye],
                          axis=1).astype(np.float32)
    assert cstA.shape == (T, _CA_TOT)

    later = np.triu(np.ones((T, T), np.float32), 1)
    cstB = np.broadcast_to(later[:, None, :], (T, R, T)).reshape(T, R * T)
    return np.ascontiguousarray(cstA), np.ascontiguousarray(cstB)


def build_nc(use_collective: bool = False, debug_outputs: bool = False):
    nc = bacc.Bacc("TRN2", target_bir_lowering=False, debug=False,
                   num_devices=NCORES)

    tg16_d = nc.dram_tensor("tg16", [T, 16], F32, kind="ExternalInput")
    awh_d = nc.dram_tensor("awh", [1, 72], F32, kind="ExternalInput")
    outcat_d = nc.dram_tensor("outcat", [OUTCAT_ELEMS], F32, kind="ExternalInput")
    loss_d = nc.dram_tensor("loss", [1, 1], F32, kind="ExternalOutput")
    cstA_np, cstB_np = _consts()
    cstA_d = nc.inline_tensor(cstA_np, name="cstA")
    cstB_d = nc.inline_tensor(cstB_np, name="cstB")

    dbg = {}
    if debug_outputs:
        for nm, shape in [("d_key", [T, 12]), ("d_ov", [T, 12]),
                          ("d_m", [T, 12]), ("d_anc", [T, 12]),
                          ("d_overlap", [T, 12]), ("d_S", [T, 24]),
                          ("d_sums", [1, 24]), ("d_gath", [T, 48]),
                          ("d_idx", [T, 12]), ("d_t4", [T, 48]),
                          ("d_fxy", [T, 24])]:
            dbg[nm] = nc.dram_tensor(nm, shape, F32, kind="ExternalOutput")

    AL = mybir.AluOpType
    AX = mybir.AxisListType.X

    with tile.TileContext(nc) as tc:
        with (
            tc.tile_pool(name="sbuf", bufs=1) as sp,
            tc.tile_pool(name="psum", bufs=1, space="PSUM") as pp,
            tc.tile_pool(name="dram", bufs=1, space="DRAM") as dp,
        ):
            def tt(out, in0, in1, op):
                nc.vector.tensor_tensor(out=out, in0=in0, in1=in1, op=op)

            def ts(out, in0, s1, op, s2=None, op2=None):
                if op2 is None:
                    nc.vector.tensor_scalar(out=out, in0=in0, scalar1=s1,
                                            scalar2=None, op0=op)
                else:
                    nc.vector.tensor_scalar(out=out, in0=in0, scalar1=s1,
                                            scalar2=s2, op0=op, op1=op2)

            def stt(out, in0, scalar, in1, op0, op1):
                nc.vector.scalar_tensor_tensor(out=out, in0=in0, scalar=scalar,
                                               in1=in1, op0=op0, op1=op1)

            _tn = [0]

            def new(shape, dt=F32):
                _tn[0] += 1
                return sp.tile(shape, dt, name=f"t{_tn[0]}")

            # ---------- loads ----------
            tgt = new([T, 16])
            nc.sync.dma_start(out=tgt[:], in_=tg16_d[:, :])
            awhT = new([T, 72])
            nc.sync.dma_start(out=awhT[:], in_=awh_d[:, :].to_broadcast([T, 72]))
            cstA = new([T, _CA_TOT])
            nc.sync.dma_start(out=cstA[:], in_=cstA_d[:, :])
            lat = new([T, R * T])
            nc.scalar.dma_start(out=lat[:], in_=cstB_d[:, :])

            def C(c0, w):
                return cstA[:, c0:c0 + w]

            ones = new([T, 1])
            nc.vector.memset(ones[:], 1.0)

            # ---------- validity (dep: tgt only; runs during cstA load) ---
            sv = new([T, 4])
            nc.vector.reduce_sum(out=sv[:],
                                 in_=tgt[:].rearrange("p (b c) -> p b c", c=4),
                                 axis=AX)
            v4 = new([T, 4]); ts(v4[:], sv[:], 0.0, AL.is_gt)

            # ---------- t = raw * g ----------
            t4 = new([T, 48])
            tt(t4[:], tgt[:, None, :].to_broadcast([T, 3, 16]), C(_C_G4, 48),
               AL.mult)
            t4v = t4[:].rearrange("p (r c) -> p r c", c=4)
            txy = t4v[:, :, 0:2]
            twh = t4v[:, :, 2:4]

            # ---------- floor(xy): round-magic + fixup ----------
            r1 = new([T, 24])
            ts(r1[:], txy, float(2 ** 23), AL.add)
            r2 = new([T, 24])
            ts(r2[:], r1[:], -float(2 ** 23), AL.add)
            gtm = new([T, 24])
            tt(gtm[:], r2[:], txy, AL.is_gt)
            fxy = new([T, 24])
            tt(fxy[:], r2[:], gtm[:], AL.subtract)
            fv = fxy[:].rearrange("p (r q) -> p r q", q=2)
            cx4 = fv[:, :, 0:1]
            cy4 = fv[:, :, 1:2]

            # ---------- target rect ----------
            zt05 = new([T, 24])
            stt(zt05[:], txy, -0.5, fxy[:], AL.add, AL.subtract)
            twhh = new([T, 24])
            ts(twhh[:], twh, 0.5, AL.mult)
            lo = new([T, 24]); tt(lo[:], zt05[:], twhh[:], AL.subtract)
            hi = new([T, 24]); tt(hi[:], zt05[:], twhh[:], AL.add)

            # ---------- anchors + IoU in (q, r, a) layout ----------
            awhh = new([T, 72]); ts(awhh[:], awhT[:], 0.5, AL.mult)
            nawhh = new([T, 72]); ts(nawhh[:], awhh[:], -1.0, AL.mult)
            areaa = new([T, 36])
            tt(areaa[:], awhT[:, 0:36], awhT[:, 36:72], AL.mult)

            def bc72(t24):
                return (t24[:].rearrange("p (r q) -> p q r", q=2)[:, :, :, None]
                        .to_broadcast([T, 2, 12, 3]))

            P0 = new([T, 72]); tt(P0[:], bc72(lo), nawhh[:], AL.max)
            P1 = new([T, 72]); tt(P1[:], bc72(hi), awhh[:], AL.min)
            D = new([T, 72]); tt(D[:], P1[:], P0[:], AL.subtract)
            FG = new([T, 72]); ts(FG[:], D[:], 0.0, AL.is_gt)
            flag = new([T, 36]); tt(flag[:], FG[:, 0:36], FG[:, 36:72], AL.mult)
            pre = new([T, 36]); tt(pre[:], D[:, 0:36], D[:, 36:72], AL.mult)
            inter = new([T, 36]); tt(inter[:], pre[:], flag[:], AL.mult)
            dT = new([T, 24]); tt(dT[:], hi[:], lo[:], AL.subtract)
            dv = dT[:].rearrange("p (r q) -> p r q", q=2)
            areat = new([T, 12]); tt(areat[:], dv[:, :, 0:1], dv[:, :, 1:2], AL.mult)
            un1 = new([T, 36])
            tt(un1[:], areat[:, :, None].to_broadcast([T, 12, 3]), areaa[:], AL.add)
            union = new([T, 36]); tt(union[:], un1[:], inter[:], AL.subtract)
            runi = new([T, 36]); nc.vector.reciprocal(out=runi[:], in_=union[:])
            iou = new([T, 36]); tt(iou[:], inter[:], runi[:], AL.mult)

            # ---------- overlap / argmax ----------
            overlap = new([T, 12])
            nc.vector.reduce_max(out=overlap[:],
                                 in_=iou[:].rearrange("p (r a) -> p r a", a=3),
                                 axis=AX)
            iv = iou[:].rearrange("p (r a) -> p r a", a=3)
            eq0 = new([T, 12]); tt(eq0[:], iv[:, :, 0:1], overlap[:], AL.is_equal)
            eq1 = new([T, 12]); tt(eq1[:], iv[:, :, 1:2], overlap[:], AL.is_equal)
            t2 = new([T, 12]); ts(t2[:], eq1[:], 0.0, AL.is_equal, 1.0, AL.add)
            neq0 = new([T, 12]); ts(neq0[:], eq0[:], 0.0, AL.is_equal)
            anc = new([T, 12]); tt(anc[:], neq0[:], t2[:], AL.mult)

            # ---------- cell + gather offsets (critical path head) ----------
            ca = new([T, 12]); tt(ca[:], anc[:], C(_C_HW, 12), AL.mult)
            cb = new([T, 12]); tt(cb[:], cy4, C(_C_W, 12), AL.mult)
            cc = new([T, 12]); tt(cc[:], ca[:], cb[:], AL.add)
            cell = new([T, 12]); tt(cell[:], cc[:], cx4, AL.add)
            idf = new([T, 12])
            stt(idf[:], cell[:], 85.0, C(_C_BGOFF, 12), AL.mult, AL.add)
            idxi = new([T, 12], I32)
            nc.vector.tensor_copy(out=idxi[:], in_=idf[:])

            # indices to [100,6]: partition p = bh*50+j, col q = s*2+bl
            idx2 = new([100, 6], I32)
            iview = idxi[:].rearrange("p (s bh bl) -> p s bh bl", bh=2, bl=2)
            for bh in range(2):
                nc.scalar.dma_start(out=idx2[bh * 50:(bh + 1) * 50, :],
                                    in_=iview[:, :, bh, :])

            # 6 indirect gathers (HW: one index per dst partition)
            gath2 = new([100, 24])
            for q in range(6):
                nc.gpsimd.indirect_dma_start(
                    out=gath2[:, q * 4:(q + 1) * 4], out_offset=None,
                    in_=outcat_d[:].unsqueeze(1),
                    in_offset=bass.IndirectOffsetOnAxis(ap=idx2[:, q:q + 1],
                                                        axis=0),
                )
            gath = new([T, 48])
            gview = gath[:].rearrange("p (s u) -> p s u", u=16)
            for bh in range(2):
                nc.scalar.dma_start(out=gview[:, :, bh * 8:(bh + 1) * 8],
                                    in_=gath2[bh * 50:(bh + 1) * 50, :])

            # ---------- gather-independent work (fills the gather window) --
            rct = new([T, 48]); nc.vector.reciprocal(out=rct[:], in_=t4[:])
            rst = new([T, 48]); nc.scalar.sqrt(out=rst[:], in_=rct[:])

            om = new([T, 12]); ts(om[:], overlap[:], 0.5, AL.is_gt)
            m = new([T, 12])
            tt(m[:], om[:].rearrange("p (s b) -> p s b", b=4),
               v4[:, None, :].to_broadcast([T, 3, 4]), AL.mult)

            # ---------- dedup key + last-wins winner ----------
            kk = new([T, 12])
            stt(kk[:], cell[:], -SENT, m[:], AL.add, AL.mult)
            key = new([T, 12]); tt(key[:], kk[:], C(_C_JCR, 12), AL.add)

            keyT_p = pp.tile([R, T], F32, name="keyT_p")
            nc.tensor.matmul(out=keyT_p[:], lhsT=key[:], rhs=C(_C_EYE, T),
                             start=True, stop=True)
            keyT = new([R, T])
            nc.vector.tensor_copy(out=keyT[:], in_=keyT_p[:])
            kd2 = nc.dram_tensor("kd2", [R * T], F32)
            nc.sync.dma_start(out=kd2[:].rearrange("(r k) -> r k", k=T),
                              in_=keyT[:])
            keyB = new([T, R * T])
            nc.sync.dma_start(out=keyB[:],
                              in_=kd2[:].unsqueeze(0).to_broadcast([T, R * T]))

            E = new([T, R * T])
            tt(E[:], key[:, :, None].to_broadcast([T, 12, T]), keyB[:],
               AL.is_equal)
            EL = new([T, R * T])
            tt(EL[:], E[:], lat[:], AL.mult)
            ov = new([T, 12])
            nc.vector.reduce_max(out=ov[:],
                                 in_=EL[:].rearrange("p (r k) -> p r k", k=T),
                                 axis=AX)
            S = new([T, 24])
            nov = new([T, 12]); ts(nov[:], ov[:], 0.0, AL.is_equal)
            tt(S[:, 0:12], m[:], nov[:], AL.mult)          # winner

            # ---------- gather-dependent loss terms ----------
            rcp = new([T, 48]); nc.vector.reciprocal(out=rcp[:], in_=gath[:])
            rsp = new([T, 48]); nc.scalar.sqrt(out=rsp[:], in_=rcp[:])
            dP = new([T, 48]); tt(dP[:], gath[:], t4[:], AL.subtract)
            dR = new([T, 48]); tt(dR[:], rsp[:], rst[:], AL.subtract)
            dd = new([T, 48]); tt(dd[:], dP[:], dR[:], AL.subtract)
            md = new([T, 48]); tt(md[:], C(_C_MXY, 48), dd[:], AL.mult)
            sel = new([T, 48]); tt(sel[:], dR[:], md[:], AL.add)
            sq = new([T, 48]); tt(sq[:], sel[:], sel[:], AL.mult)
            termsum = new([T, 12])
            nc.vector.reduce_sum(out=termsum[:],
                                 in_=sq[:].rearrange("p (r c) -> p r c", c=4),
                                 axis=AX)
            tt(S[:, 12:24], S[:, 0:12], termsum[:], AL.mult)

            # ---------- per-row normalize + total ----------
            sums_p = pp.tile([1, 24], F32, name="sums_p")
            nc.tensor.matmul(out=sums_p[:], lhsT=ones[:], rhs=S[:],
                             start=True, stop=True)
            sums = new([1, 24])
            nc.vector.tensor_copy(out=sums[:], in_=sums_p[:])
            mx2 = new([1, 12])
            ts(mx2[:], sums[:, 0:12], 1.0, AL.max, 2.0, AL.mult)
            rden = new([1, 12]); nc.vector.reciprocal(out=rden[:], in_=mx2[:])
            rl = new([1, 12]); tt(rl[:], sums[:, 12:24], rden[:], AL.mult)
            part = new([1, 1])
            nc.vector.reduce_sum(out=part[:], in_=rl[:], axis=AX)
            p32 = new([1, 1])
            ts(p32[:], part[:], 1.0 / B_TOTAL, AL.mult)

            if debug_outputs:
                for nm, src in [("d_key", key), ("d_ov", ov), ("d_m", m),
                                ("d_anc", anc), ("d_overlap", overlap),
                                ("d_S", S), ("d_sums", sums),
                                ("d_gath", gath), ("d_idx", idf),
                                ("d_t4", t4), ("d_fxy", fxy)]:
                    nc.sync.dma_start(out=dbg[nm][:, :], in_=src[:])

            if use_collective:
                ccin = dp.tile([1, 1], F32, name="ccin")
                ccout = dp.tile([1, 1], F32, name="ccout")
                nc.sync.dma_start(out=ccin[:], in_=p32[:])
                nc.gpsimd.collective_compute(
                    "AllReduce", AL.add,
                    replica_groups=[list(range(NCORES))],
                    ins=[ccin[:].opt()], outs=[ccout[:].opt()],
                )
                nc.sync.dma_start(out=loss_d[:, :], in_=ccout[:])
            else:
                nc.sync.dma_start(out=loss_d[:, :], in_=p32[:])

    nc.compile()
    return nc


def make_in_maps(output0, anchors0, output1, anchors1, output2, anchors2,
                 targets):
    outs = [np.asarray(output0), np.asarray(output1), np.asarray(output2)]
    ancs = [np.asarray(anchors0), np.asarray(anchors1), np.asarray(anchors2)]
    tg = np.asarray(targets)

    aw = np.concatenate([np.tile(a[:, 0], PB) for a in ancs])  # [36] (r, a)
    ah = np.concatenate([np.tile(a[:, 1], PB) for a in ancs])
    awh = np.concatenate([aw, ah]).astype(np.float32)[None, :]  # [1,72]

    in_maps = []
    for c in range(NCORES):
        sl = slice(c * PB, (c + 1) * PB)
        tg16 = np.ascontiguousarray(
            tg[sl, :, 1:5].transpose(1, 0, 2).reshape(T, 16).astype(np.float32))
        outcat = np.concatenate([o[sl].ravel() for o in outs]).astype(np.float32)
        in_maps.append({"tg16": tg16, "awh": awh, "outcat": outcat})
    return in_maps


_NC_CACHE = {}


def kernel(output0, anchors0, output1, anchors1, output2, anchors2, targets):
    from concourse.bass_utils import run_bass_kernel_spmd

    if "nc" not in _NC_CACHE:
        _NC_CACHE["nc"] = build_nc(use_collective=False)
    nc = _NC_CACHE["nc"]
    in_maps = make_in_maps(output0, anchors0, output1, anchors1, output2,
                           anchors2, targets)
    res = run_bass_kernel_spmd(nc, in_maps, list(range(NCORES)))
    total = np.float32(0.0)
    for c in range(NCORES):
        total += np.float32(res.results[c]["loss"].reshape(()))
    return np.float32(total)


# revision 16
# speedup vs baseline: 2.0552x; 1.0399x over previous
"""Trainium2 Bass kernel for nn_BoxLoss (YOLO-style box regression loss).

Contract: kernel(**inputs) takes FULL unsharded inputs (numpy), returns the
FULL scalar loss. Internally: pure data parallel over batch across 8
NeuronCores (4 images per core); each core computes its 12 (scale, image)
row losses entirely on-device and writes its partial sum; the host adds
the 8 partials while unsharding.

Only ~50 targets x 12 rows of real work exist per core; the big
[B,A,g,g,85] activation tensors are touched ONLY via indirect (gather)
DMAs of the <=600 matched cells x 4 channels the loss actually reads -
the kernel never streams the full tensors.

Device layout: partition dim = target j (50), free dim r = s*4 + b
(3 scales x 4 local images = 12 rows), innermost c (4 box channels).
The gather runs in a [100, *] layout (partition = (b-half, j)) so each
indirect DMA consumes one index per partition (the HW semantics).
"""

import numpy as np

import concourse.bass as bass
import concourse.bacc as bacc
import concourse.mybir as mybir
import concourse.tile as tile

NCORES = 8
GRIDS = (52, 26, 13)
A = 3           # anchors per scale
T = 50          # targets per image
PB = 4          # images per core
R = 3 * PB      # (scale, image) rows per core
BLOCK = 8192.0  # per-row key offset; cells < 3*52*52 = 8112 < 8192
SENT = 8112.0   # sentinel cell id for unmatched targets (>= any real cell)
B_TOTAL = 32

F32 = mybir.dt.float32
I32 = mybir.dt.int32

_SCALE_ELEMS = [PB * A * g * g * 85 for g in GRIDS]
_SCALE_BASE = [0, _SCALE_ELEMS[0], _SCALE_ELEMS[0] + _SCALE_ELEMS[1]]
OUTCAT_ELEMS = sum(_SCALE_ELEMS)

# cstA column layout
_C_JCR = 0        # [0,12)    8112 + r*8192
_C_BGOFF = 12     # [12,24)   scale base + b*3*g^2*85
_C_HW = 24        # [24,36)   g^2
_C_W = 36         # [36,48)   g
_C_G4 = 48        # [48,96)   g per (s,b,c)
_C_MXY = 96       # [96,144)  1 for xy channels
_C_EYE = 144      # [144,194) identity 50x50
_CA_TOT = 194


def _consts():
    r = np.arange(R, dtype=np.float32)[None, :]
    s = (r // PB).astype(np.int64)
    b = (r % PB).astype(np.int64)
    g = np.array(GRIDS, dtype=np.float32)[s]

    jcr = np.broadcast_to(SENT + r * BLOCK, (T, R))
    base = np.array(_SCALE_BASE, dtype=np.float64)[s]
    bgoff = np.broadcast_to(base + b * (A * 85) * (g.astype(np.float64) ** 2),
                            (T, R)).astype(np.float32)
    hw4 = np.broadcast_to(g * g, (T, R))
    w4 = np.broadcast_to(g, (T, R))
    g4 = np.broadcast_to(g[:, :, None], (T, R, 4)).reshape(T, 48)
    mxy = np.broadcast_to(np.array([1, 1, 0, 0], np.float32), (T, R, 4)).reshape(T, 48)
    eye = np.eye(T, dtype=np.float32)
    cstA = np.concatenate([jcr, bgoff, hw4, w4, g4, mxy, eye],
                          axis=1).astype(np.float32)
    assert cstA.shape == (T, _CA_TOT)

    later = np.triu(np.ones((T, T), np.float32), 1)
    cstB = np.broadcast_to(later[:, None, :], (T, R, T)).reshape(T, R * T)
    return np.ascontiguousarray(cstA), np.ascontiguousarray(cstB)


def build_nc(use_collective: bool = False, debug_outputs: bool = False):
    nc = bacc.Bacc("TRN2", target_bir_lowering=False, debug=False,
                   num_devices=NCORES)

    tg16_d = nc.dram_tensor("tg16", [T, 16], F32, kind="ExternalInput")
    awh_d = nc.dram_tensor("awh", [1, 72], F32, kind="ExternalInput")
    outcat_d = nc.dram_tensor("outcat", [OUTCAT_ELEMS], F32, kind="ExternalInput")
    loss_d = nc.dram_tensor("loss", [1, 1], F32, kind="ExternalOutput")
    cstA_np, cstB_np = _consts()
    cstA_d = nc.inline_tensor(cstA_np, name="cstA")
    cstB_d = nc.inline_tensor(cstB_np, name="cstB")

    dbg = {}
    if debug_outputs:
        for nm, shape in [("d_key", [T, 12]), ("d_ov", [T, 12]),
                          ("d_m", [T, 12]), ("d_anc", [T, 12]),
                          ("d_overlap", [T, 12]), ("d_S", [T, 24]),
                          ("d_sums", [1, 24]), ("d_gath", [T, 48]),
                          ("d_idx", [T, 12]), ("d_t4", [T, 48]),
                          ("d_fxy", [T, 24])]:
            dbg[nm] = nc.dram_tensor(nm, shape, F32, kind="ExternalOutput")

    AL = mybir.AluOpType
    AX = mybir.AxisListType.X

    with tile.TileContext(nc) as tc:
        with (
            tc.tile_pool(name="sbuf", bufs=1) as sp,
            tc.tile_pool(name="psum", bufs=1, space="PSUM") as pp,
            tc.tile_pool(name="dram", bufs=1, space="DRAM") as dp,
        ):
            def tt(out, in0, in1, op):
                nc.vector.tensor_tensor(out=out, in0=in0, in1=in1, op=op)

            def ts(out, in0, s1, op, s2=None, op2=None):
                if op2 is None:
                    nc.vector.tensor_scalar(out=out, in0=in0, scalar1=s1,
                                            scalar2=None, op0=op)
                else:
                    nc.vector.tensor_scalar(out=out, in0=in0, scalar1=s1,
                                            scalar2=s2, op0=op, op1=op2)

            def stt(out, in0, scalar, in1, op0, op1):
                nc.vector.scalar_tensor_tensor(out=out, in0=in0, scalar=scalar,
                                               in1=in1, op0=op0, op1=op1)

            _tn = [0]

            def new(shape, dt=F32):
                _tn[0] += 1
                return sp.tile(shape, dt, name=f"t{_tn[0]}")

            # ---------- loads ----------
            tgt = new([T, 16])
            nc.sync.dma_start(out=tgt[:], in_=tg16_d[:, :])
            awhT = new([T, 72])
            nc.sync.dma_start(out=awhT[:], in_=awh_d[:, :].to_broadcast([T, 72]))
            cstA = new([T, _CA_TOT])
            nc.sync.dma_start(out=cstA[:], in_=cstA_d[:, :])
            lat = new([T, R * T])
            nc.scalar.dma_start(out=lat[:], in_=cstB_d[:, :])

            def C(c0, w):
                return cstA[:, c0:c0 + w]

            ones = new([T, 1])
            nc.vector.memset(ones[:], 1.0)

            # ---------- validity (dep: tgt only; runs during cstA load) ---
            sv = new([T, 4])
            nc.vector.reduce_sum(out=sv[:],
                                 in_=tgt[:].rearrange("p (b c) -> p b c", c=4),
                                 axis=AX)
            v4 = new([T, 4]); ts(v4[:], sv[:], 0.0, AL.is_gt)

            # ---------- t = raw * g ----------
            t4 = new([T, 48])
            tt(t4[:], tgt[:, None, :].to_broadcast([T, 3, 16]), C(_C_G4, 48),
               AL.mult)
            t4v = t4[:].rearrange("p (r c) -> p r c", c=4)
            txy = t4v[:, :, 0:2]
            twh = t4v[:, :, 2:4]

            # ---------- floor(xy): round-magic + fixup ----------
            r1 = new([T, 24])
            ts(r1[:], txy, float(2 ** 23), AL.add)
            r2 = new([T, 24])
            ts(r2[:], r1[:], -float(2 ** 23), AL.add)
            gtm = new([T, 24])
            tt(gtm[:], r2[:], txy, AL.is_gt)
            fxy = new([T, 24])
            tt(fxy[:], r2[:], gtm[:], AL.subtract)
            fv = fxy[:].rearrange("p (r q) -> p r q", q=2)
            cx4 = fv[:, :, 0:1]
            cy4 = fv[:, :, 1:2]

            # ---------- target rect ----------
            zt05 = new([T, 24])
            stt(zt05[:], txy, -0.5, fxy[:], AL.add, AL.subtract)
            lo = new([T, 24])
            stt(lo[:], twh, -0.5, zt05[:], AL.mult, AL.add)
            hi = new([T, 24])
            stt(hi[:], twh, 0.5, zt05[:], AL.mult, AL.add)

            # ---------- anchors + IoU in (q, r, a) layout ----------
            awhh = new([T, 72]); ts(awhh[:], awhT[:], 0.5, AL.mult)
            nawhh = new([T, 72]); ts(nawhh[:], awhT[:], -0.5, AL.mult)
            areaa = new([T, 36])
            tt(areaa[:], awhT[:, 0:36], awhT[:, 36:72], AL.mult)

            def bc72(t24):
                return (t24[:].rearrange("p (r q) -> p q r", q=2)[:, :, :, None]
                        .to_broadcast([T, 2, 12, 3]))

            P0 = new([T, 72]); tt(P0[:], bc72(lo), nawhh[:], AL.max)
            P1 = new([T, 72]); tt(P1[:], bc72(hi), awhh[:], AL.min)
            # inter = max(x1-x0,0) * max(y1-y0,0)  (== dx*dy*flag exactly)
            D = new([T, 72]); tt(D[:], P1[:], P0[:], AL.subtract)
            M0 = new([T, 72]); ts(M0[:], D[:], 0.0, AL.max)
            inter = new([T, 36]); tt(inter[:], M0[:, 0:36], M0[:, 36:72], AL.mult)
            dT = new([T, 24]); tt(dT[:], hi[:], lo[:], AL.subtract)
            dv = dT[:].rearrange("p (r q) -> p r q", q=2)
            areat = new([T, 12]); tt(areat[:], dv[:, :, 0:1], dv[:, :, 1:2], AL.mult)
            un1 = new([T, 36])
            tt(un1[:], areat[:, :, None].to_broadcast([T, 12, 3]), areaa[:], AL.add)
            union = new([T, 36]); tt(union[:], un1[:], inter[:], AL.subtract)
            runi = new([T, 36]); nc.vector.reciprocal(out=runi[:], in_=union[:])
            iou = new([T, 36]); tt(iou[:], inter[:], runi[:], AL.mult)

            # ---------- overlap / argmax ----------
            overlap = new([T, 12])
            nc.vector.reduce_max(out=overlap[:],
                                 in_=iou[:].rearrange("p (r a) -> p r a", a=3),
                                 axis=AX)
            iv = iou[:].rearrange("p (r a) -> p r a", a=3)
            eq0 = new([T, 12]); tt(eq0[:], iv[:, :, 0:1], overlap[:], AL.is_equal)
            eq1 = new([T, 12]); tt(eq1[:], iv[:, :, 1:2], overlap[:], AL.is_equal)
            t2 = new([T, 12]); ts(t2[:], eq1[:], 0.0, AL.is_equal, 1.0, AL.add)
            neq0 = new([T, 12]); ts(neq0[:], eq0[:], 0.0, AL.is_equal)
            anc = new([T, 12]); tt(anc[:], neq0[:], t2[:], AL.mult)

            # ---------- cell + gather offsets (critical path head) ----------
            ca = new([T, 12]); tt(ca[:], anc[:], C(_C_HW, 12), AL.mult)
            cb = new([T, 12]); tt(cb[:], cy4, C(_C_W, 12), AL.mult)
            cc = new([T, 12]); tt(cc[:], ca[:], cb[:], AL.add)
            cell = new([T, 12]); tt(cell[:], cc[:], cx4, AL.add)
            idf = new([T, 12])
            stt(idf[:], cell[:], 85.0, C(_C_BGOFF, 12), AL.mult, AL.add)
            idxi = new([T, 12], I32)
            nc.vector.tensor_copy(out=idxi[:], in_=idf[:])

            # indices to [100,6]: partition p = bh*50+j, col q = s*2+bl
            idx2 = new([100, 6], I32)
            iview = idxi[:].rearrange("p (s bh bl) -> p s bh bl", bh=2, bl=2)
            nc.scalar.dma_start(out=idx2[0:50, :], in_=iview[:, :, 0, :])
            nc.sync.dma_start(out=idx2[50:100, :], in_=iview[:, :, 1, :])

            # 6 indirect gathers (HW: one index per dst partition), each
            # stripe copied back as soon as its gather lands (2 rings).
            gath2 = new([100, 24])
            gath = new([T, 48])
            gview = gath[:].rearrange("p (s bh bl c) -> p s bh bl c",
                                      bh=2, bl=2, c=4)
            for q in range(6):
                s_, bl = q // 2, q % 2
                nc.gpsimd.indirect_dma_start(
                    out=gath2[:, q * 4:(q + 1) * 4], out_offset=None,
                    in_=outcat_d[:].unsqueeze(1),
                    in_offset=bass.IndirectOffsetOnAxis(ap=idx2[:, q:q + 1],
                                                        axis=0),
                )
                for bh in range(2):
                    eng = nc.scalar if bh == 0 else nc.sync
                    eng.dma_start(
                        out=gview[:, s_, bh, bl, :],
                        in_=gath2[bh * 50:(bh + 1) * 50, q * 4:(q + 1) * 4])

            # ---------- gather-independent work (fills the gather window) --
            rct = new([T, 48]); nc.vector.reciprocal(out=rct[:], in_=t4[:])
            rst = new([T, 48]); nc.scalar.sqrt(out=rst[:], in_=rct[:])

            om = new([T, 12]); ts(om[:], overlap[:], 0.5, AL.is_gt)
            m = new([T, 12])
            tt(m[:], om[:].rearrange("p (s b) -> p s b", b=4),
               v4[:, None, :].to_broadcast([T, 3, 4]), AL.mult)

            # ---------- dedup key + last-wins winner ----------
            kk = new([T, 12])
            stt(kk[:], cell[:], -SENT, m[:], AL.add, AL.mult)
            key = new([T, 12]); tt(key[:], kk[:], C(_C_JCR, 12), AL.add)

            keyT_p = pp.tile([R, T], F32, name="keyT_p")
            nc.tensor.matmul(out=keyT_p[:], lhsT=key[:], rhs=C(_C_EYE, T),
                             start=True, stop=True)
            keyT = new([R, T])
            nc.vector.tensor_copy(out=keyT[:], in_=keyT_p[:])
            kd2 = nc.dram_tensor("kd2", [R * T], F32)
            nc.sync.dma_start(out=kd2[:].rearrange("(r k) -> r k", k=T),
                              in_=keyT[:])
            keyB = new([T, R * T])
            nc.sync.dma_start(out=keyB[:],
                              in_=kd2[:].unsqueeze(0).to_broadcast([T, R * T]))

            E = new([T, R * T])
            tt(E[:], key[:, :, None].to_broadcast([T, 12, T]), keyB[:],
               AL.is_equal)
            EL = new([T, R * T])
            tt(EL[:], E[:], lat[:], AL.mult)
            ov = new([T, 12])
            nc.vector.reduce_max(out=ov[:],
                                 in_=EL[:].rearrange("p (r k) -> p r k", k=T),
                                 axis=AX)
            S = new([T, 24])
            nov = new([T, 12]); ts(nov[:], ov[:], 0.0, AL.is_equal)
            tt(S[:, 0:12], m[:], nov[:], AL.mult)          # winner

            # ---------- gather-dependent loss terms ----------
            rcp = new([T, 48]); nc.vector.reciprocal(out=rcp[:], in_=gath[:])
            rsp = new([T, 48]); nc.scalar.sqrt(out=rsp[:], in_=rcp[:])
            dP = new([T, 48]); tt(dP[:], gath[:], t4[:], AL.subtract)
            dR = new([T, 48]); tt(dR[:], rsp[:], rst[:], AL.subtract)
            dd = new([T, 48]); tt(dd[:], dP[:], dR[:], AL.subtract)
            md = new([T, 48]); tt(md[:], C(_C_MXY, 48), dd[:], AL.mult)
            sel = new([T, 48]); tt(sel[:], dR[:], md[:], AL.add)
            sq = new([T, 48]); tt(sq[:], sel[:], sel[:], AL.mult)
            termsum = new([T, 12])
            nc.vector.reduce_sum(out=termsum[:],
                                 in_=sq[:].rearrange("p (r c) -> p r c", c=4),
                                 axis=AX)
            tt(S[:, 12:24], S[:, 0:12], termsum[:], AL.mult)

            # ---------- per-row normalize + total ----------
            sums_p = pp.tile([1, 24], F32, name="sums_p")
            nc.tensor.matmul(out=sums_p[:], lhsT=ones[:], rhs=S[:],
                             start=True, stop=True)
            sums = new([1, 24])
            nc.vector.tensor_copy(out=sums[:], in_=sums_p[:])
            mx2 = new([1, 12])
            ts(mx2[:], sums[:, 0:12], 1.0, AL.max, 2.0, AL.mult)
            rden = new([1, 12]); nc.vector.reciprocal(out=rden[:], in_=mx2[:])
            rl = new([1, 12]); tt(rl[:], sums[:, 12:24], rden[:], AL.mult)
            part = new([1, 1])
            nc.vector.reduce_sum(out=part[:], in_=rl[:], axis=AX)
            p32 = new([1, 1])
            ts(p32[:], part[:], 1.0 / B_TOTAL, AL.mult)

            if debug_outputs:
                for nm, src in [("d_key", key), ("d_ov", ov), ("d_m", m),
                                ("d_anc", anc), ("d_overlap", overlap),
                                ("d_S", S), ("d_sums", sums),
                                ("d_gath", gath), ("d_idx", idf),
                                ("d_t4", t4), ("d_fxy", fxy)]:
                    nc.sync.dma_start(out=dbg[nm][:, :], in_=src[:])

            if use_collective:
                ccin = dp.tile([1, 1], F32, name="ccin")
                ccout = dp.tile([1, 1], F32, name="ccout")
                nc.sync.dma_start(out=ccin[:], in_=p32[:])
                nc.gpsimd.collective_compute(
                    "AllReduce", AL.add,
                    replica_groups=[list(range(NCORES))],
                    ins=[ccin[:].opt()], outs=[ccout[:].opt()],
                )
                nc.sync.dma_start(out=loss_d[:, :], in_=ccout[:])
            else:
                nc.sync.dma_start(out=loss_d[:, :], in_=p32[:])

    nc.compile()
    return nc


def make_in_maps(output0, anchors0, output1, anchors1, output2, anchors2,
                 targets):
    outs = [np.asarray(output0), np.asarray(output1), np.asarray(output2)]
    ancs = [np.asarray(anchors0), np.asarray(anchors1), np.asarray(anchors2)]
    tg = np.asarray(targets)

    aw = np.concatenate([np.tile(a[:, 0], PB) for a in ancs])  # [36] (r, a)
    ah = np.concatenate([np.tile(a[:, 1], PB) for a in ancs])
    awh = np.concatenate([aw, ah]).astype(np.float32)[None, :]  # [1,72]

    in_maps = []
    for c in range(NCORES):
        sl = slice(c * PB, (c + 1) * PB)
        tg16 = np.ascontiguousarray(
            tg[sl, :, 1:5].transpose(1, 0, 2).reshape(T, 16).astype(np.float32))
        outcat = np.concatenate([o[sl].ravel() for o in outs]).astype(np.float32)
        in_maps.append({"tg16": tg16, "awh": awh, "outcat": outcat})
    return in_maps


_NC_CACHE = {}


def kernel(output0, anchors0, output1, anchors1, output2, anchors2, targets):
    from concourse.bass_utils import run_bass_kernel_spmd

    if "nc" not in _NC_CACHE:
        _NC_CACHE["nc"] = build_nc(use_collective=False)
    nc = _NC_CACHE["nc"]
    in_maps = make_in_maps(output0, anchors0, output1, anchors1, output2,
                           anchors2, targets)
    res = run_bass_kernel_spmd(nc, in_maps, list(range(NCORES)))
    total = np.float32(0.0)
    for c in range(NCORES):
        total += np.float32(res.results[c]["loss"].reshape(()))
    return np.float32(total)


# revision 17
# speedup vs baseline: 2.2261x; 1.0832x over previous
"""Trainium2 Bass kernel for nn_BoxLoss (YOLO-style box regression loss).

Contract: kernel(**inputs) takes FULL unsharded inputs (numpy), returns the
FULL scalar loss. Internally: pure data parallel over batch across 8
NeuronCores (4 images per core); each core computes its 12 (scale, image)
row losses entirely on-device and writes its partial sum; the host adds
the 8 partials while unsharding.

Only ~50 targets x 12 rows of real work exist per core; the big
[B,A,g,g,85] activation tensors are touched ONLY via indirect (gather)
DMAs of the <=600 matched cells x 4 channels the loss actually reads -
the kernel never streams the full tensors.

Layouts:
  matching math   [50, *]  partition = target j, free r = s*4 + b
  gather + loss   [100, *] partition = (b-half, j), free q = s*2 + bl
The indirect-DMA HW consumes ONE index per destination partition, so the
[100,*] layout needs only 6 gathers; the per-scale loss chains run inside
the gather window and the final reduction stays partition-local (PE
matmuls with block-indicator lhsT), so nothing crosses partitions after
the last gather.
"""

import numpy as np

import concourse.bass as bass
import concourse.bacc as bacc
import concourse.mybir as mybir
import concourse.tile as tile

NCORES = 8
GRIDS = (52, 26, 13)
A = 3           # anchors per scale
T = 50          # targets per image
PB = 4          # images per core
R = 3 * PB      # (scale, image) rows per core
BLOCK = 8192.0  # per-row key offset; cells < 3*52*52 = 8112 < 8192
SENT = 8112.0   # sentinel cell id for unmatched targets (>= any real cell)
B_TOTAL = 32

F32 = mybir.dt.float32
I32 = mybir.dt.int32

_SCALE_ELEMS = [PB * A * g * g * 85 for g in GRIDS]
_SCALE_BASE = [0, _SCALE_ELEMS[0], _SCALE_ELEMS[0] + _SCALE_ELEMS[1]]
OUTCAT_ELEMS = sum(_SCALE_ELEMS)

# cstA column layout ([50, _CA_TOT])
_C_G4 = 0         # [0,48)    g per (s,b,c)
_C_JCR = 48       # [48,60)   8112 + r*8192
_C_BGOFF = 60     # [60,72)   scale base + b*3*g^2*85
_C_HW = 72        # [72,84)   g^2
_C_W = 84         # [84,96)   g
_C_EYE = 96       # [96,146)  identity 50x50
_CA_TOT = 146

# cst100 column layout ([100, 10])
_D_ONESU = 0      # [0,2)   block indicator: col u = 1 if p//50 == u
_D_MXY8 = 2       # [2,10)  [1,1,0,0,1,1,0,0]


def _consts():
    r = np.arange(R, dtype=np.float32)[None, :]
    s = (r // PB).astype(np.int64)
    b = (r % PB).astype(np.int64)
    g = np.array(GRIDS, dtype=np.float32)[s]

    g4 = np.broadcast_to(g[:, :, None], (T, R, 4)).reshape(T, 48)
    jcr = np.broadcast_to(SENT + r * BLOCK, (T, R))
    base = np.array(_SCALE_BASE, dtype=np.float64)[s]
    bgoff = np.broadcast_to(base + b * (A * 85) * (g.astype(np.float64) ** 2),
                            (T, R)).astype(np.float32)
    hw4 = np.broadcast_to(g * g, (T, R))
    w4 = np.broadcast_to(g, (T, R))
    eye = np.eye(T, dtype=np.float32)
    cstA = np.concatenate([g4, jcr, bgoff, hw4, w4, eye],
                          axis=1).astype(np.float32)
    assert cstA.shape == (T, _CA_TOT)

    later = np.triu(np.ones((T, T), np.float32), 1)
    cstB = np.ascontiguousarray(
        np.broadcast_to(later[:, None, :], (T, R, T)).reshape(T, R * T))

    onesu = np.zeros((100, 2), np.float32)
    onesu[0:50, 0] = 1.0
    onesu[50:100, 1] = 1.0
    mxy8 = np.broadcast_to(np.array([1, 1, 0, 0], np.float32), (100, 2, 4))
    cst100 = np.concatenate([onesu, mxy8.reshape(100, 8)],
                            axis=1).astype(np.float32)
    return np.ascontiguousarray(cstA), cstB, np.ascontiguousarray(cst100)


def build_nc(use_collective: bool = False):
    nc = bacc.Bacc("TRN2", target_bir_lowering=False, debug=False,
                   num_devices=NCORES)

    tg16_d = nc.dram_tensor("tg16", [T, 16], F32, kind="ExternalInput")
    awh_d = nc.dram_tensor("awh", [1, 72], F32, kind="ExternalInput")
    outcat_d = nc.dram_tensor("outcat", [OUTCAT_ELEMS], F32, kind="ExternalInput")
    loss_d = nc.dram_tensor("loss", [1, 1], F32, kind="ExternalOutput")
    cstA_np, cstB_np, cst100_np = _consts()
    cstA_d = nc.inline_tensor(cstA_np, name="cstA")
    cstB_d = nc.inline_tensor(cstB_np, name="cstB")
    cst100_d = nc.inline_tensor(cst100_np, name="cst100")

    AL = mybir.AluOpType
    AX = mybir.AxisListType.X

    with tile.TileContext(nc) as tc:
        with (
            tc.tile_pool(name="sbuf", bufs=1) as sp,
            tc.tile_pool(name="psum", bufs=1, space="PSUM") as pp,
            tc.tile_pool(name="dram", bufs=1, space="DRAM") as dp,
        ):
            def tt(out, in0, in1, op):
                nc.vector.tensor_tensor(out=out, in0=in0, in1=in1, op=op)

            def ts(out, in0, s1, op, s2=None, op2=None):
                if op2 is None:
                    nc.vector.tensor_scalar(out=out, in0=in0, scalar1=s1,
                                            scalar2=None, op0=op)
                else:
                    nc.vector.tensor_scalar(out=out, in0=in0, scalar1=s1,
                                            scalar2=s2, op0=op, op1=op2)

            def stt(out, in0, scalar, in1, op0, op1):
                nc.vector.scalar_tensor_tensor(out=out, in0=in0, scalar=scalar,
                                               in1=in1, op0=op0, op1=op1)

            _tn = [0]

            def new(shape, dt=F32):
                _tn[0] += 1
                return sp.tile(shape, dt, name=f"t{_tn[0]}")

            # ---------- loads ----------
            tgt = new([T, 16])
            nc.sync.dma_start(out=tgt[:], in_=tg16_d[:, :])
            awhT = new([T, 72])
            nc.sync.dma_start(out=awhT[:], in_=awh_d[:, :].to_broadcast([T, 72]))
            cstA = new([T, _CA_TOT])
            nc.sync.dma_start(out=cstA[:], in_=cstA_d[:, :])
            cstH = new([100, 10])
            nc.scalar.dma_start(out=cstH[:], in_=cst100_d[:, :])
            lat = new([T, R * T])
            nc.scalar.dma_start(out=lat[:], in_=cstB_d[:, :])

            def C(c0, w):
                return cstA[:, c0:c0 + w]

            onesU = cstH[:, _D_ONESU:_D_ONESU + 2]
            MXY8 = cstH[:, _D_MXY8:_D_MXY8 + 8]

            ones2 = new([2, 1])
            nc.vector.memset(ones2[:], 1.0)

            # ---------- validity (dep: tgt only) ----------
            sv = new([T, 4])
            nc.vector.reduce_sum(out=sv[:],
                                 in_=tgt[:].rearrange("p (b c) -> p b c", c=4),
                                 axis=AX)
            v4 = new([T, 4]); ts(v4[:], sv[:], 0.0, AL.is_gt)

            # ---------- t = raw * g ----------
            t4 = new([T, 48])
            tt(t4[:], tgt[:, None, :].to_broadcast([T, 3, 16]), C(_C_G4, 48),
               AL.mult)
            t4v = t4[:].rearrange("p (r c) -> p r c", c=4)
            txy = t4v[:, :, 0:2]
            twh = t4v[:, :, 2:4]

            # ---------- floor(xy) ----------
            r1 = new([T, 24])
            ts(r1[:], txy, float(2 ** 23), AL.add)
            r2 = new([T, 24])
            ts(r2[:], r1[:], -float(2 ** 23), AL.add)
            gtm = new([T, 24])
            tt(gtm[:], r2[:], txy, AL.is_gt)
            fxy = new([T, 24])
            tt(fxy[:], r2[:], gtm[:], AL.subtract)
            fv = fxy[:].rearrange("p (r q) -> p r q", q=2)
            cx4 = fv[:, :, 0:1]
            cy4 = fv[:, :, 1:2]

            # ---------- target rect ----------
            zt05 = new([T, 24])
            stt(zt05[:], txy, -0.5, fxy[:], AL.add, AL.subtract)
            lo = new([T, 24])
            stt(lo[:], twh, -0.5, zt05[:], AL.mult, AL.add)
            hi = new([T, 24])
            stt(hi[:], twh, 0.5, zt05[:], AL.mult, AL.add)

            # ---------- anchors + IoU in (xy?, r, a) layout ----------
            awhh = new([T, 72]); ts(awhh[:], awhT[:], 0.5, AL.mult)
            nawhh = new([T, 72]); ts(nawhh[:], awhT[:], -0.5, AL.mult)
            areaa = new([T, 36])
            tt(areaa[:], awhT[:, 0:36], awhT[:, 36:72], AL.mult)

            def bc72(t24):
                return (t24[:].rearrange("p (r q) -> p q r", q=2)[:, :, :, None]
                        .to_broadcast([T, 2, 12, 3]))

            P0 = new([T, 72]); tt(P0[:], bc72(lo), nawhh[:], AL.max)
            P1 = new([T, 72]); tt(P1[:], bc72(hi), awhh[:], AL.min)
            # inter = max(x1-x0,0)*max(y1-y0,0)  (== dx*dy*flag bit-exactly)
            D = new([T, 72]); tt(D[:], P1[:], P0[:], AL.subtract)
            M0 = new([T, 72]); ts(M0[:], D[:], 0.0, AL.max)
            inter = new([T, 36]); tt(inter[:], M0[:, 0:36], M0[:, 36:72], AL.mult)
            dT = new([T, 24]); tt(dT[:], hi[:], lo[:], AL.subtract)
            dv = dT[:].rearrange("p (r q) -> p r q", q=2)
            areat = new([T, 12]); tt(areat[:], dv[:, :, 0:1], dv[:, :, 1:2], AL.mult)
            un1 = new([T, 36])
            tt(un1[:], areat[:, :, None].to_broadcast([T, 12, 3]), areaa[:], AL.add)
            union = new([T, 36]); tt(union[:], un1[:], inter[:], AL.subtract)
            runi = new([T, 36]); nc.vector.reciprocal(out=runi[:], in_=union[:])
            iou = new([T, 36]); tt(iou[:], inter[:], runi[:], AL.mult)

            # ---------- overlap / argmax / cell / gather offsets ----------
            overlap = new([T, 12])
            nc.vector.reduce_max(out=overlap[:],
                                 in_=iou[:].rearrange("p (r a) -> p r a", a=3),
                                 axis=AX)
            iv = iou[:].rearrange("p (r a) -> p r a", a=3)
            eq0 = new([T, 12]); tt(eq0[:], iv[:, :, 0:1], overlap[:], AL.is_equal)
            eq1 = new([T, 12]); tt(eq1[:], iv[:, :, 1:2], overlap[:], AL.is_equal)
            t2 = new([T, 12]); ts(t2[:], eq1[:], 0.0, AL.is_equal, 1.0, AL.add)
            neq0 = new([T, 12]); ts(neq0[:], eq0[:], 0.0, AL.is_equal)
            anc = new([T, 12]); tt(anc[:], neq0[:], t2[:], AL.mult)

            ca = new([T, 12]); tt(ca[:], anc[:], C(_C_HW, 12), AL.mult)
            cb = new([T, 12]); tt(cb[:], cy4, C(_C_W, 12), AL.mult)
            cc = new([T, 12]); tt(cc[:], ca[:], cb[:], AL.add)
            cell = new([T, 12]); tt(cell[:], cc[:], cx4, AL.add)
            idf = new([T, 12])
            stt(idf[:], cell[:], 85.0, C(_C_BGOFF, 12), AL.mult, AL.add)
            idxi = new([T, 12], I32)
            nc.vector.tensor_copy(out=idxi[:], in_=idf[:])

            # indices to [100,6]: partition p = bh*50+j, col q = s*2+bl
            idx2 = new([100, 6], I32)
            iview = idxi[:].rearrange("p (s bh bl) -> p s bh bl", bh=2, bl=2)
            nc.scalar.dma_start(out=idx2[0:50, :], in_=iview[:, :, 0, :])
            nc.sync.dma_start(out=idx2[50:100, :], in_=iview[:, :, 1, :])

            # ---------- dedup (runs while gathers execute) ----------
            om = new([T, 12]); ts(om[:], overlap[:], 0.5, AL.is_gt)
            m = new([T, 12])
            tt(m[:], om[:].rearrange("p (s b) -> p s b", b=4),
               v4[:, None, :].to_broadcast([T, 3, 4]), AL.mult)
            kk = new([T, 12])
            stt(kk[:], cell[:], -SENT, m[:], AL.add, AL.mult)
            key = new([T, 12]); tt(key[:], kk[:], C(_C_JCR, 12), AL.add)

            keyT_p = pp.tile([R, T], F32, name="keyT_p")
            nc.tensor.matmul(out=keyT_p[:], lhsT=key[:], rhs=C(_C_EYE, T),
                             start=True, stop=True)
            keyT = new([R, T])
            nc.vector.tensor_copy(out=keyT[:], in_=keyT_p[:])
            kd2 = nc.dram_tensor("kd2", [R * T], F32)
            nc.sync.dma_start(out=kd2[:].rearrange("(r k) -> r k", k=T),
                              in_=keyT[:])
            keyB = new([T, R * T])
            nc.sync.dma_start(out=keyB[:],
                              in_=kd2[:].unsqueeze(0).to_broadcast([T, R * T]))

            # ---------- t in gather layout + rsqrt(t) (early) ----------
            t42 = new([100, 24])
            tv = t4[:].rearrange("p (s u) -> p s u", u=16)
            nc.scalar.dma_start(out=t42[0:50, :], in_=tv[:, :, 0:8])
            nc.sync.dma_start(out=t42[50:100, :], in_=tv[:, :, 8:16])
            rct2 = new([100, 24]); nc.vector.reciprocal(out=rct2[:], in_=t42[:])
            rst2 = new([100, 24]); nc.scalar.sqrt(out=rst2[:], in_=rct2[:])

            # ---------- 6 indirect gathers + per-scale loss chains ----------
            gath2 = new([100, 24])
            for q in range(6):
                nc.gpsimd.indirect_dma_start(
                    out=gath2[:, q * 4:(q + 1) * 4], out_offset=None,
                    in_=outcat_d[:].unsqueeze(1),
                    in_offset=bass.IndirectOffsetOnAxis(ap=idx2[:, q:q + 1],
                                                        axis=0),
                )

            TS2 = new([100, 6])
            winner = new([T, 12])
            winner2 = new([100, 6])

            def stripe_chain(s_):
                cols = slice(s_ * 8, (s_ + 1) * 8)
                g8 = gath2[:, cols]
                t8 = t42[:, cols]
                rs8 = rst2[:, cols]
                rcp = new([100, 8]); nc.vector.reciprocal(out=rcp[:], in_=g8)
                rsp = new([100, 8]); nc.scalar.sqrt(out=rsp[:], in_=rcp[:])
                dP = new([100, 8]); tt(dP[:], g8, t8, AL.subtract)
                dR = new([100, 8]); tt(dR[:], rsp[:], rs8, AL.subtract)
                dd = new([100, 8]); tt(dd[:], dP[:], dR[:], AL.subtract)
                md = new([100, 8]); tt(md[:], MXY8, dd[:], AL.mult)
                sel = new([100, 8]); tt(sel[:], dR[:], md[:], AL.add)
                sq = new([100, 8]); tt(sq[:], sel[:], sel[:], AL.mult)
                nc.vector.reduce_sum(
                    out=TS2[:, 2 * s_:2 * s_ + 2],
                    in_=sq[:].rearrange("p (bl c) -> p bl c", c=4), axis=AX)

            # scale 0 chain (ready first)
            stripe_chain(0)

            # dedup tail -> winner, relocation, n / rden (gather window)
            E = new([T, R * T])
            tt(E[:], key[:, :, None].to_broadcast([T, 12, T]), keyB[:],
               AL.is_equal)
            EL = new([T, R * T])
            tt(EL[:], E[:], lat[:], AL.mult)
            ov = new([T, 12])
            nc.vector.reduce_max(out=ov[:],
                                 in_=EL[:].rearrange("p (r k) -> p r k", k=T),
                                 axis=AX)
            nov = new([T, 12]); ts(nov[:], ov[:], 0.0, AL.is_equal)
            tt(winner[:], m[:], nov[:], AL.mult)
            wv = winner[:].rearrange("p (s bh bl) -> p s bh bl", bh=2, bl=2)
            nc.scalar.dma_start(out=winner2[0:50, :], in_=wv[:, :, 0, :])
            nc.sync.dma_start(out=winner2[50:100, :], in_=wv[:, :, 1, :])

            stripe_chain(1)

            n2_p = pp.tile([2, 6], F32, name="n2_p")
            nc.tensor.matmul(out=n2_p[:], lhsT=onesU, rhs=winner2[:],
                             start=True, stop=True)
            mx2 = new([2, 6])
            ts(mx2[:], n2_p[:], 1.0, AL.max, 2.0, AL.mult)
            rden2 = new([2, 6]); nc.vector.reciprocal(out=rden2[:], in_=mx2[:])

            stripe_chain(2)

            # ---------- partition-local final reduction ----------
            wt2 = new([100, 6]); tt(wt2[:], TS2[:], winner2[:], AL.mult)
            wls_p = pp.tile([2, 6], F32, name="wls_p")
            nc.tensor.matmul(out=wls_p[:], lhsT=onesU, rhs=wt2[:],
                             start=True, stop=True)
            rl2 = new([2, 6]); tt(rl2[:], wls_p[:], rden2[:], AL.mult)
            pt2 = new([2, 1])
            nc.vector.reduce_sum(out=pt2[:], in_=rl2[:], axis=AX)
            tot_p = pp.tile([1, 1], F32, name="tot_p")
            nc.tensor.matmul(out=tot_p[:], lhsT=ones2[:], rhs=pt2[:],
                             start=True, stop=True)
            p32 = new([1, 1])
            ts(p32[:], tot_p[:], 1.0 / B_TOTAL, AL.mult)

            if use_collective:
                ccin = dp.tile([1, 1], F32, name="ccin")
                ccout = dp.tile([1, 1], F32, name="ccout")
                nc.sync.dma_start(out=ccin[:], in_=p32[:])
                nc.gpsimd.collective_compute(
                    "AllReduce", AL.add,
                    replica_groups=[list(range(NCORES))],
                    ins=[ccin[:].opt()], outs=[ccout[:].opt()],
                )
                nc.sync.dma_start(out=loss_d[:, :], in_=ccout[:])
            else:
                nc.sync.dma_start(out=loss_d[:, :], in_=p32[:])

    nc.compile()
    return nc


def make_in_maps(output0, anchors0, output1, anchors1, output2, anchors2,
                 targets):
    outs = [np.asarray(output0), np.asarray(output1), np.asarray(output2)]
    ancs = [np.asarray(anchors0), np.asarray(anchors1), np.asarray(anchors2)]
    tg = np.asarray(targets)

    aw = np.concatenate([np.tile(a[:, 0], PB) for a in ancs])  # [36] (r, a)
    ah = np.concatenate([np.tile(a[:, 1], PB) for a in ancs])
    awh = np.concatenate([aw, ah]).astype(np.float32)[None, :]  # [1,72]

    in_maps = []
    for c in range(NCORES):
        sl = slice(c * PB, (c + 1) * PB)
        tg16 = np.ascontiguousarray(
            tg[sl, :, 1:5].transpose(1, 0, 2).reshape(T, 16).astype(np.float32))
        outcat = np.concatenate([o[sl].ravel() for o in outs]).astype(np.float32)
        in_maps.append({"tg16": tg16, "awh": awh, "outcat": outcat})
    return in_maps


_NC_CACHE = {}


def kernel(output0, anchors0, output1, anchors1, output2, anchors2, targets):
    from concourse.bass_utils import run_bass_kernel_spmd

    if "nc" not in _NC_CACHE:
        _NC_CACHE["nc"] = build_nc(use_collective=False)
    nc = _NC_CACHE["nc"]
    in_maps = make_in_maps(output0, anchors0, output1, anchors1, output2,
                           anchors2, targets)
    res = run_bass_kernel_spmd(nc, in_maps, list(range(NCORES)))
    total = np.float32(0.0)
    for c in range(NCORES):
        total += np.float32(res.results[c]["loss"].reshape(()))
    return np.float32(total)
